# revision 1
# baseline (speedup 1.0000x reference)
"""CSAEncoder Trainium2 kernel: 3-branch cross-attention + concat DoubleConv.

Sharding (8 cores): 2 batch groups x 4 tensor ranks.
Core c: batch b = c // 4, rank g = c % 4.
  - Attention: core computes heads [4g, 4g+4) of all 3 branches for batch b
    (a contiguous 128-channel slab of each branch's output).
  - conv1 computed as partial sums over the core's local 384 input channels
    for ALL 512 output channels; per-branch bf16 AllReduce(add) within the
    4-core batch group (branches 0/1 overlap later attention; branch 2 is
    split into two channel chunks pipelined against h1+conv2).
  - conv2 computed locally: full 512-channel contraction, only the core's own
    128 output channels. No further collective.
Host assembles the full (2, 512, 32, 32) output from the 8 per-core slabs.

v2 changes vs v1:
  - BN bias of each attention branch folded into the v/o projection bias
    host-side (y + b*denom = sum_k (u+b) p), so the post-softmax division is
    a single tensor_mul per head.
  - Softmax denominators: 1/d computed as exp(-ln d) on ACT (ln and exp
    share one activation-table set; see the get_activation_tables patch) +
    gpsimd partition_broadcast (SBUF->SBUF) instead of the 6.5ns/elem DVE
    reciprocal + DMA roundtrip through DRAM.  (The custom-DVE
    reciprocal_approx_fast op produces garbage on this hardware.)
  - Per-branch AllReduce in bf16 instead of one fp32 AllReduce at the end.
  - Consolidated input DMAs; projections interleaved into branch-0 slots.
  - AllReduce-dependent DMAs (art loads) are kept OFF the gpsimd and vector
    engine streams: the Tile scheduler otherwise hoists them ahead of the
    branch-2 softmax broadcasts/muls in the engine FIFO, serializing the
    whole pipeline behind a slow collective.
"""

import os
import sys

import ml_dtypes
import numpy as np

for _p in ("/opt/trn_rl_repo",):
    if _p not in sys.path and os.path.isdir(_p):
        sys.path.insert(0, _p)

import concourse.bass as bass
import concourse.mybir as mybir
import concourse.tile as tile
from concourse import bacc
from concourse.bass_utils import run_bass_kernel_spmd

F32 = mybir.dt.float32
BF16 = mybir.dt.bfloat16
AF = mybir.ActivationFunctionType

# The ACT table-set picker is greedy-first-match: with both Exp and Ln in the
# kernel it alternates exp_and_others <-> natural_log (~2.7us per reload, ~25
# reloads).  Restrict matching to the one set that contains every function we
# use (exp, ln, relu, copy) so exactly one table load is emitted.  Keyed by
# name, dict length/order preserved so set ids stay valid.
_ACT_KEEP_SET = "natural_log_exp_and_others"
_orig_get_act_tables = bacc.get_activation_tables


def _patched_get_act_tables(arch):
    tabs = _orig_get_act_tables(arch)
    return {n: (fns if n == _ACT_KEEP_SET else set()) for n, fns in tabs.items()}


bacc.get_activation_tables = _patched_get_act_tables
B, C, H, W, HEADS = 2, 512, 32, 32, 16
D = C // HEADS            # 32
S = H * W                 # 1024
EPS = 1e-5
ISQD = 1.0 / np.sqrt(D)   # folded into the exp activation
NCORES = 8
GROUPS = [[0, 1, 2, 3], [4, 5, 6, 7]]
HP = W + 2                # padded row stride (34)


def build_nc():
    nc = bacc.Bacc(None, target_bir_lowering=False)

    # ---- per-core external inputs -------------------------------------
    x4_d = nc.declare_dram_parameter("x4", [4, 128, S], BF16, isOutput=False)
    oth_d = nc.declare_dram_parameter("oth", [2, 4, 128, S], BF16, isOutput=False)
    wqT_d = nc.declare_dram_parameter("wqT", [3, 4, 128, 128], BF16, isOutput=False)
    wkT_d = nc.declare_dram_parameter("wkT", [3, 4, 128, 128], BF16, isOutput=False)
    wvoT_d = nc.declare_dram_parameter("wvoT", [4, 128, 384], BF16, isOutput=False)
    dvec_d = nc.declare_dram_parameter("dvec", [128, 6], F32, isOutput=False)
    wobv_d = nc.declare_dram_parameter("wobv", [1, 384], F32, isOutput=False)
    c1wT_d = nc.declare_dram_parameter("c1wT", [3, 4, 128, 9, 128], BF16, isOutput=False)
    c2wT_d = nc.declare_dram_parameter("c2wT", [4, 128, 9, 128], BF16, isOutput=False)
    avec_d = nc.declare_dram_parameter("avec", [128, 10], F32, isOutput=False)
    out_d = nc.declare_dram_parameter("out", [128, S], F32, isOutput=True)

    with tile.TileContext(nc) as tc:
        import contextlib

        ctx = contextlib.ExitStack()
        with ctx:
            const = ctx.enter_context(tc.tile_pool(name="const", bufs=1))
            kq = ctx.enter_context(tc.tile_pool(name="kq", bufs=1))
            xtp = ctx.enter_context(tc.tile_pool(name="xtp", bufs=1))
            stg = ctx.enter_context(tc.tile_pool(name="stg", bufs=1))
            brp = ctx.enter_context(tc.tile_pool(name="brp", bufs=2))
            rcp = ctx.enter_context(tc.tile_pool(name="rcp", bufs=2))
            scps = ctx.enter_context(tc.tile_pool(name="scps", bufs=2, space="PSUM"))
            yps = ctx.enter_context(tc.tile_pool(name="yps", bufs=1, space="PSUM"))
            smps = ctx.enter_context(tc.tile_pool(name="smps", bufs=2, space="PSUM"))
            dram = ctx.enter_context(tc.tile_pool(name="dram", bufs=1, space="DRAM"))

            # ---- activations + branch-0 weights first (DMA priority) -----
            x_sb = const.tile([128, 4, S], BF16)
            nc.sync.dma_start(out=x_sb, in_=x4_d[:].rearrange("k p s -> p k s"))
            wq_sb = const.tile([128, 3, 4, 128], BF16)
            wk_sb = const.tile([128, 3, 4, 128], BF16)
            nc.sync.dma_start(out=wk_sb, in_=wkT_d[:].rearrange("i k p f -> p i k f"))
            nc.sync.dma_start(out=wq_sb, in_=wqT_d[:].rearrange("i k p f -> p i k f"))
            othp = ctx.enter_context(tc.tile_pool(name="othp", bufs=1))
            oth1 = othp.tile([128, 4, S], BF16, name="oth")
            nc.sync.dma_start(out=oth1, in_=oth_d[0].rearrange("k p s -> p k s"))
            wvo_sb = const.tile([128, 4, 384], BF16)
            nc.sync.dma_start(out=wvo_sb, in_=wvoT_d[:].rearrange("k p f -> p k f"))

            # Small consts: DMA to staging, then re-own on the consuming
            # engine (DVE / ACT) so consumers need no cross-engine const wait.
            dvec_st = const.tile([128, 6], F32)
            nc.gpsimd.dma_start(out=dvec_st, in_=dvec_d[:])
            wobv_st = const.tile([128, 384], F32)
            nc.gpsimd.dma_start(out=wobv_st, in_=wobv_d[:].partition_broadcast(128))
            avec_st = const.tile([128, 10], F32)
            nc.gpsimd.dma_start(out=avec_st, in_=avec_d[:])
            dvec = const.tile([128, 6], F32)
            nc.vector.tensor_copy(dvec, dvec_st)
            wobv_sb = const.tile([128, 384], F32)
            nc.vector.tensor_copy(wobv_sb, wobv_st)
            avec = const.tile([128, 10], F32)
            nc.scalar.activation(out=avec, in_=avec_st, func=AF.Copy)
            bqv_sb = dvec[:, 0:3]
            bkv_sb = dvec[:, 3:6]

            # xt (attention output) slabs + h1 slabs, zero-padded 34x34
            xt_sl = []
            for i in range(3):
                t = xtp.tile([128, HP, HP], BF16, name=f"xt{i}")
                nc.vector.memset(t, 0.0)
                xt_sl.append(t)
            h1_sl = []
            for k in range(4):
                t = xtp.tile([128, HP, HP], BF16, name=f"h1{k}")
                nc.vector.memset(t, 0.0)
                h1_sl.append(t)
            # per-branch conv1 partials (f32 staging for the AllReduce);
            # 2 rotating slots — branch i is shipped before i+1 is written
            brst = {}

            def get_brst(i):
                if i not in brst:
                    brst[i] = brp.tile([128, 4, S], BF16, name="brst")
                return brst[i]

            # Semaphore warmers: absorb const-DMA + memset waits into each
            # engine's observed clock so later compute ops need <=1 wait.
            warm = const.tile([128, 1], F32)
            nc.vector.tensor_copy(warm, dvec[:, 0:1])
            warm2 = const.tile([128, 1], F32)
            nc.scalar.activation(out=warm2, in_=warm, func=AF.Copy)

            # k/q per branch (with biases added), uT tiles
            k_sb = kq.tile([128, 3, S], BF16)
            q_sb = kq.tile([128, 3, S], BF16)
            uT = [kq.tile([128, 3, 4, 33], BF16, name=f"uT{t}") for t in range(8)]

            # ---- projections (k0/q0 upfront; rest fill branch-0 slots) ---
            # x_prev reuses x_next's slot once q_proj(0) has consumed it
            qsrc = [oth1, x_sb, None]

            def load_oth2():
                oth2 = othp.tile([128, 4, S], BF16, name="oth")
                nc.sync.dma_start(out=oth2, in_=oth_d[1].rearrange("k p s -> p k s"))
                qsrc[2] = oth2

            def k_proj(i):
                k_ps = scps.tile([128, S], F32, name="kq_ps", tag="sc")
                for s in range(2):
                    for ks in range(4):
                        nc.tensor.matmul(
                            k_ps[:, 512 * s : 512 * (s + 1)],
                            lhsT=wk_sb[:, i, ks, :],
                            rhs=x_sb[:, ks, 512 * s : 512 * (s + 1)],
                            start=(ks == 0),
                            stop=(ks == 3),
                        )
                nc.vector.tensor_scalar_add(k_sb[:, i, :], k_ps, bkv_sb[:, i : i + 1])

            def q_proj(i):
                q_ps = scps.tile([128, S], F32, name="kq_ps2", tag="sc")
                for s in range(2):
                    for ks in range(4):
                        nc.tensor.matmul(
                            q_ps[:, 512 * s : 512 * (s + 1)],
                            lhsT=wq_sb[:, i, ks, :],
                            rhs=qsrc[i][:, ks, 512 * s : 512 * (s + 1)],
                            start=(ks == 0),
                            stop=(ks == 3),
                        )
                nc.vector.tensor_scalar_add(q_sb[:, i, :], q_ps, bqv_sb[:, i : i + 1])

            def u_proj(t):
                u_ps = smps.tile([128, 384], F32, name="u_ps", tag="sm")
                for ks in range(4):
                    nc.tensor.matmul(
                        u_ps,
                        lhsT=x_sb[:, ks, 128 * t : 128 * (t + 1)],
                        rhs=wvo_sb[:, ks, :],
                        start=(ks == 0),
                        stop=(ks == 3),
                    )
                nc.vector.memset(uT[t][:, :, :, 32:33], 1.0)
                # wobv has the attention-BN bias folded in host-side
                nc.vector.tensor_add(
                    uT[t][:, :, :, 0:32],
                    u_ps.rearrange("p (i h d) -> p i h d", i=3, h=4),
                    wobv_sb.rearrange("p (i h d) -> p i h d", i=3, h=4),
                )

            k_proj(0)
            q_proj(0)
            load_oth2()

            # ---- conv weights (emitted after proj psum freed) ------------
            convw = ctx.enter_context(tc.tile_pool(name="convw", bufs=1))
            pt = ctx.enter_context(tc.tile_pool(name="pt", bufs=16))
            c1w_sb = [
                [convw.tile([128, 9, 128], BF16, name=f"c1w{i}_{m}") for m in range(4)]
                for i in range(3)
            ]
            for i in range(3):
                for m in range(4):
                    nc.sync.dma_start(out=c1w_sb[i][m], in_=c1wT_d[i, m])
            c2w_sb = [convw.tile([128, 9, 128], BF16, name=f"c2w{k}") for k in range(4)]
            for k in range(4):
                nc.sync.dma_start(out=c2w_sb[k], in_=c2wT_d[k])

            def conv1_block(i, m, n):
                """Partial conv1 for xt slab i, out m-tile, spatial half n,
                written (bf16) into brst[i]."""
                ps = smps.tile([128, 512], F32, name="cv", tag="sm")
                for dy in range(3):
                    for dx in range(3):
                        nc.tensor.matmul(
                            ps,
                            lhsT=c1w_sb[i][m][:, dy * 3 + dx, :],
                            rhs=xt_sl[i][:, 16 * n + dy : 16 * n + dy + 16, dx : dx + 32],
                            start=(dy == 0 and dx == 0),
                            stop=(dy == 2 and dx == 2),
                        )
                nc.vector.tensor_copy(get_brst(i)[:, m, 512 * n : 512 * (n + 1)], ps)
                if n == 1:
                    if i < 2:
                        dst = partial[i][128 * m : 128 * (m + 1), :]
                    else:
                        dst = partial2[m // 2][128 * (m % 2) : 128 * (m % 2) + 128, :]
                    nc.gpsimd.dma_start(out=dst, in_=get_brst(i)[:, m, :])

            def attention(i, pr, filler):
                """Heads (2pr, 2pr+1) of branch i.  `filler` is a list of
                thunks (conv1 blocks / projections) sprinkled between the
                per-t score groups to keep PE dense while ACT grinds exps."""
                heads = (2 * pr, 2 * pr + 1)
                pts = {}
                fi = 0
                for t in range(8):
                    for h in heads:
                        sc = scps.tile([128, S], F32, name="sc", tag="sc")
                        p0 = 32 * h
                        for s in range(2):
                            nc.tensor.matmul(
                                sc[:, 512 * s : 512 * (s + 1)],
                                lhsT=k_sb[p0 : p0 + 32, i, 128 * t : 128 * (t + 1)],
                                rhs=q_sb[p0 : p0 + 32, i, 512 * s : 512 * (s + 1)],
                                start=True,
                                stop=True,
                                tile_position=(p0, 0),
                            )
                        ptt = pt.tile([128, S], BF16, name="ptt")
                        nc.scalar.activation(
                            out=ptt, in_=sc, func=AF.Exp, scale=float(ISQD)
                        )
                        pts[(h, t)] = ptt
                    while fi < len(filler) * (t + 1) // 8:
                        filler[fi]()
                        fi += 1
                # y chains: per head, the two query-half chains target the
                # two zero regions of one [33, S] psum tile sequentially
                for h in heads:
                    y2 = yps.tile([33, S], F32, name="y2", tag="y")
                    for s in range(2):
                        for t in range(8):
                            nc.tensor.matmul(
                                y2[:, 512 * s : 512 * (s + 1)],
                                lhsT=uT[t][:, i, h, :],
                                rhs=pts[(h, t)][:, 512 * s : 512 * (s + 1)],
                                start=(t == 0),
                                stop=(t == 7),
                            )
                    p0 = 32 * h
                    # 1/denom as exp(-ln(denom)) on ACT: ln and exp share the
                    # natural_log_exp_and_others table set (no reload), and
                    # both DVE reciprocal (6.5ns/elem) and the custom-DVE
                    # approx op (garbage on this HW) are avoided.
                    rc = rcp.tile([1, S], F32, name="rc")
                    nc.scalar.activation(out=rc, in_=y2[32:33, :], func=AF.Ln)
                    nc.scalar.activation(out=rc, in_=rc, func=AF.Exp, scale=-1.0)
                    rcb = rcp.tile([32, S], F32, name="rcb")
                    nc.gpsimd.partition_broadcast(rcb, rc[:])
                    nc.vector.tensor_mul(
                        xt_sl[i][p0 : p0 + 32, 1:33, 1:33],
                        y2[0:32, :].rearrange("p (a b) -> p a b", b=32),
                        rcb.rearrange("p (a b) -> p a b", b=32),
                    )
                while fi < len(filler):
                    filler[fi]()
                    fi += 1

            # ---- collectives ---------------------------------------------
            # branch 0/1: one [512, S] bf16 AllReduce each, overlapped with
            # the next branch's attention.  branch 2: two [256, S] chunks
            # (m01 / m23) pipelined against h1+conv2.
            partial = [dram.tile([512, S], BF16, name=f"part{i}") for i in range(2)]
            art = [dram.tile([512, S], BF16, name=f"art{i}") for i in range(2)]
            partial2 = [dram.tile([256, S], BF16, name=f"part2{a}") for a in range(2)]
            art2 = [dram.tile([256, S], BF16, name=f"art2{a}") for a in range(2)]

            def ar_branch(i):
                nc.gpsimd.collective_compute(
                    "AllReduce",
                    mybir.AluOpType.add,
                    replica_groups=GROUPS,
                    ins=[partial[i][:]],
                    outs=[art[i][:]],
                )

            def ar2_chunk(a):
                nc.gpsimd.collective_compute(
                    "AllReduce",
                    mybir.AluOpType.add,
                    replica_groups=GROUPS,
                    ins=[partial2[a][:]],
                    outs=[art2[a][:]],
                )

            def conv1_and_ship(i):
                return [
                    (lambda m=m, n=n: conv1_block(i, m, n))
                    for m in range(4)
                    for n in range(2)
                ]

            # ---- phase A: attention with projections/conv1 interleaved ---
            attention(0, 0, [lambda t=t: u_proj(t) for t in range(8)])
            attention(0, 1, [lambda: k_proj(1), lambda: q_proj(1),
                             lambda: k_proj(2), lambda: q_proj(2)])
            f0 = conv1_and_ship(0)
            attention(1, 0, f0[:4])
            attention(1, 1, f0[4:] + [lambda: ar_branch(0)])
            f1 = conv1_and_ship(1)
            attention(2, 0, f1[:4])
            attention(2, 1, f1[4:] + [lambda: ar_branch(1)])
            # branch 2 conv1 + chunked AR at the end
            for m in range(4):
                for n in range(2):
                    conv1_block(2, m, n)
                if m == 1:
                    ar2_chunk(0)
            ar2_chunk(1)

            # ---- phase C: combine, BN1+relu, conv2, BN2+relu, out --------
            arr01 = stg.tile([128, 4, S], BF16, name="arr01", bufs=1)
            arrt = [stg.tile([128, 2, S], BF16, name=f"arrt{j}", bufs=1) for j in range(2)]
            # art0 + art1 (during late attention / AR2 flight)
            nc.sync.dma_start(
                out=arr01, in_=art[0][:].rearrange("(m p) s -> p m s", p=128)
            )
            nc.sync.dma_start(
                out=arrt[0],
                in_=art[1][:].rearrange("(m p) s -> p m s", p=128)[:, 0:2, :],
            )
            nc.sync.dma_start(
                out=arrt[1],
                in_=art[1][:].rearrange("(m p) s -> p m s", p=128)[:, 2:4, :],
            )
            nc.vector.tensor_add(
                arr01[:, 0:2, :], arr01[:, 0:2, :], arrt[0][:, 0:2, :]
            )
            nc.vector.tensor_add(
                arr01[:, 2:4, :], arr01[:, 2:4, :], arrt[1][:, 0:2, :]
            )

            oout = stg.tile([128, S], F32, name="oout", bufs=1)
            ps2 = [smps.tile([128, 512], F32, name=f"cv2_{n}", tag="sm") for n in range(2)]

            def h1_chunk(a):
                """arr01[m01/m23] + art2 chunk -> BN1+relu -> h1 slabs."""
                nc.sync.dma_start(
                    out=arrt[0],
                    in_=art2[a][:].rearrange("(m p) s -> p m s", p=128),
                )
                src = arrt[0]
                nc.vector.tensor_add(
                    src[:, 0:2, :], src[:, 0:2, :], arr01[:, 2 * a : 2 * a + 2, :]
                )
                for j, k in enumerate((2 * a, 2 * a + 1)):
                    nc.scalar.activation(
                        out=h1_sl[k][:, 1:33, 1:33],
                        in_=src[:, j, :].rearrange("p (a b) -> p a b", b=32),
                        func=AF.Relu,
                        bias=avec[:, 4 + k : 5 + k],
                        scale=avec[:, k : k + 1],
                    )

            def conv2_half(a):
                # accumulate k-slabs 2a, 2a+1 into both spatial halves
                for n in range(2):
                    for k in (2 * a, 2 * a + 1):
                        for dy in range(3):
                            for dx in range(3):
                                nc.tensor.matmul(
                                    ps2[n],
                                    lhsT=c2w_sb[k][:, dy * 3 + dx, :],
                                    rhs=h1_sl[k][
                                        :, 16 * n + dy : 16 * n + dy + 16, dx : dx + 32
                                    ],
                                    start=(k == 0 and dy == 0 and dx == 0),
                                    stop=(k == 3 and dy == 2 and dx == 2),
                                )

            h1_chunk(0)
            conv2_half(0)   # overlaps AR2 chunk 1
            h1_chunk(1)
            conv2_half(1)
            for n in range(2):
                nc.scalar.activation(
                    out=oout[:, 512 * n : 512 * (n + 1)],
                    in_=ps2[n],
                    func=AF.Relu,
                    bias=avec[:, 9:10],
                    scale=avec[:, 8:9],
                )
                nc.sync.dma_start(
                    out=out_d[:, 512 * n : 512 * (n + 1)],
                    in_=oout[:, 512 * n : 512 * (n + 1)],
                )

    nc.finalize()
    return nc


def _f(x):
    return np.ascontiguousarray(x, dtype=np.float32)


def _bf(x):
    return np.ascontiguousarray(np.asarray(x, dtype=np.float32).astype(ml_dtypes.bfloat16))


def prepare_core_inputs(inp):
    """Build the 8 per-core input dicts from the full-problem inputs."""
    inp = {k: np.asarray(v, dtype=np.float64) for k, v in inp.items()}
    x = inp["x"].reshape(B, C, S)
    xp = inp["x_prev"].reshape(B, C, S)
    xn = inp["x_next"].reshape(B, C, S)

    bn1s_full = inp["bn1g"] / np.sqrt(inp["bn1v"] + EPS)
    bn1b_full = inp["bn1b"] - inp["bn1m"] * bn1s_full
    bn2s_full = inp["bn2g"] / np.sqrt(inp["bn2v"] + EPS)
    bn2b_full = inp["bn2b"] - inp["bn2m"] * bn2s_full

    per_g = []
    for g in range(4):
        sl = slice(128 * g, 128 * (g + 1))
        wqT = np.stack(
            [
                np.stack([inp["Wq"][i][sl, 128 * k : 128 * (k + 1)].T for k in range(4)])
                for i in range(3)
            ]
        )
        wkT = np.stack(
            [
                np.stack([inp["Wk"][i][sl, 128 * k : 128 * (k + 1)].T for k in range(4)])
                for i in range(3)
            ]
        )
        bqv = np.stack([inp["bq"][i][sl] for i in range(3)], axis=1)
        bkv = np.stack([inp["bk"][i][sl] for i in range(3)], axis=1)

        att_s = np.stack(
            [inp["bng"][i][sl] / np.sqrt(inp["bnv"][i][sl] + EPS) for i in range(3)]
        )  # (3,128)
        xtb = np.stack(
            [
                inp["bnb"][i][sl] + (inp["bo"][i][sl] - inp["bnm"][i][sl]) * att_s[i]
                for i in range(3)
            ]
        )  # (3,128)

        wvo_rows = []
        wobv_row = []
        for i in range(3):
            for hl in range(4):
                hg = 4 * g + hl
                wv_h = inp["Wv"][i][32 * hg : 32 * (hg + 1), :]  # (32, 512)
                bv_h = inp["bv"][i][32 * hg : 32 * (hg + 1)]
                wo_h = inp["Wo"][i, hg]  # (32, 32)
                sc = att_s[i][32 * hl : 32 * (hl + 1)]  # (32,)
                wvo_rows.append(sc[:, None] * (wo_h @ wv_h))
                # fold the (BN-scaled) output bias + BN bias into the u bias:
                # y/denom + xtb == sum_k (u + xtb) p_k / denom
                wobv_row.append(sc * (wo_h @ bv_h) + xtb[i][32 * hl : 32 * (hl + 1)])
        wvo_all = np.concatenate(wvo_rows, axis=0)  # (384, 512)
        wobv = np.concatenate(wobv_row)[None, :]  # (1, 384)
        wvoT = np.stack([wvo_all[:, 128 * k : 128 * (k + 1)].T for k in range(4)])

        c1wT = np.stack(
            [
                np.stack(
                    [
                        inp["c1w"][
                            128 * m : 128 * (m + 1),
                            512 * i + 128 * g : 512 * i + 128 * (g + 1),
                        ]
                        .transpose(1, 2, 3, 0)
                        .reshape(128, 9, 128)
                        for m in range(4)
                    ]
                )
                for i in range(3)
            ]
        )
        c2wT = np.stack(
            [
                inp["c2w"][sl, 128 * k : 128 * (k + 1)]
                .transpose(1, 2, 3, 0)
                .reshape(128, 9, 128)
                for k in range(4)
            ]
        )
        avec = np.concatenate(
            [
                bn1s_full.reshape(4, 128).T,
                bn1b_full.reshape(4, 128).T,
                bn2s_full[sl][:, None],
                bn2b_full[sl][:, None],
            ],
            axis=1,
        )  # (128, 10)

        per_g.append(
            dict(
                wqT=_bf(wqT), wkT=_bf(wkT), wvoT=_bf(wvoT),
                wobv=_f(wobv), c1wT=_bf(c1wT), c2wT=_bf(c2wT),
                dvec=_f(np.concatenate([bqv, bkv], axis=1)),
                avec=_f(avec),
            )
        )

    in_maps = []
    for c in range(NCORES):
        b, g = c // 4, c % 4
        d = dict(per_g[g])
        d["x4"] = _bf(x[b].reshape(4, 128, S))
        d["oth"] = _bf(np.stack([xn[b].reshape(4, 128, S), xp[b].reshape(4, 128, S)]))
        in_maps.append(d)
    return in_maps


_NC_CACHE = {}


def get_nc():
    if "nc" not in _NC_CACHE:
        _NC_CACHE["nc"] = build_nc()
    return _NC_CACHE["nc"]


def assemble(results):
    out = np.zeros((B, C, H, W), dtype=np.float32)
    for c in range(NCORES):
        b, g = c // 4, c % 4
        out[b, 128 * g : 128 * (g + 1)] = results[c]["out"].reshape(128, H, W)
    return out


def kernel(**inputs):
    nc = get_nc()
    in_maps = prepare_core_inputs(inputs)
    res = run_bass_kernel_spmd(nc, in_maps, list(range(NCORES)))
    return assemble(res.results)



# revision 26
# speedup vs baseline: 1.0168x; 1.0168x over previous
"""CSAEncoder Trainium2 kernel v3: ACT-saturated attention + band-ReduceScatter convs.

Sharding (8 cores): 2 batch groups x 4 tensor ranks.  Core c: batch b = c // 4,
rank g = c % 4.

Attention (per core): heads [4g, 4g+4) of all 3 branches for batch b.
Processed as 6 "halves" (3 branches x head-pairs {0,2} / {1,3}):
  - exps stream t-major on ACT (the binding engine: 96 x [128,1024] exps
    ~ 110us).  2 rotating score psum tiles keep ACT fed.
  - per-half y-pass at the half boundary: 2-way column-tile-packed matmuls
    (M=33 incl. the ones-row denominator) at PE col-groups 0 and 64, emitted
    with a 2-t-slot lag into the next half so ACT never stalls.
  - reciprocals: DVE copies the two denominator rows to a compact [2,1024]
    tile; ACT does ln + exp(-x) (natural_log_exp_and_others table set);
    gpsimd broadcasts; DVE multiplies into the padded xt slab.

Convs: conv1 partials (contract own 128 ch/branch, full space) are staged
into 4 overlapping row-bands of 10 rows (8-row band + 1-row halo, zero rows
at the image edges) and ReduceScattered per branch: the RS chunk routing
delivers each rank ITS band fully reduced, halo included -- rank-dependent
band selection without breaking SPMD.  conv2 then runs fully local on the
band (all 512 out channels, N=256 matmuls), output is banded: core owns
out[b, :, 8g:8g+8, :].
"""

import os
import sys

import ml_dtypes
import numpy as np

for _p in ("/opt/trn_rl_repo",):
    if _p not in sys.path and os.path.isdir(_p):
        sys.path.insert(0, _p)

import concourse.bass as bass
import concourse.mybir as mybir
import concourse.tile as tile
from concourse import bacc
from concourse.bass_utils import run_bass_kernel_spmd

F32 = mybir.dt.float32
BF16 = mybir.dt.bfloat16
AF = mybir.ActivationFunctionType

# Restrict ACT table matching to the one set containing exp, ln, relu, copy
# so exactly one table load is emitted (see baseline kernel notes).
_ACT_KEEP_SET = "natural_log_exp_and_others"
_orig_get_act_tables = bacc.get_activation_tables


def _patched_get_act_tables(arch):
    tabs = _orig_get_act_tables(arch)
    return {n: (fns if n == _ACT_KEEP_SET else set()) for n, fns in tabs.items()}


bacc.get_activation_tables = _patched_get_act_tables

B, C, H, W, HEADS = 2, 512, 32, 32, 16
D = C // HEADS            # 32
S = H * W                 # 1024
EPS = 1e-5
ISQD = 1.0 / np.sqrt(D)
NCORES = 8
GROUPS = [[0, 1, 2, 3], [4, 5, 6, 7]]
HP = W + 2                # padded row stride for xt slabs (34)
BR = 10                   # band rows incl 1-row halo each side
BW = 8                    # band rows owned


def build_nc():
    nc = bacc.Bacc(None, target_bir_lowering=False)

    # ---- per-core external inputs -------------------------------------
    x4_d = nc.declare_dram_parameter("x4", [4, 128, S], BF16, isOutput=False)
    oth_d = nc.declare_dram_parameter("oth", [2, 4, 128, S], BF16, isOutput=False)
    wqT_d = nc.declare_dram_parameter("wqT", [3, 4, 128, 128], BF16, isOutput=False)
    wkT_d = nc.declare_dram_parameter("wkT", [3, 4, 128, 128], BF16, isOutput=False)
    wvoT_d = nc.declare_dram_parameter("wvoT", [4, 128, 384], BF16, isOutput=False)
    dvec_d = nc.declare_dram_parameter("dvec", [128, 6], F32, isOutput=False)
    wobv_d = nc.declare_dram_parameter("wobv", [1, 384], F32, isOutput=False)
    c1wT_d = nc.declare_dram_parameter("c1wT", [3, 4, 128, 9, 128], BF16, isOutput=False)
    # full conv2 weights: [k-chunk, m-chunk, 128 in-part, 9 taps, 128 out]
    c2wT_d = nc.declare_dram_parameter("c2wT", [4, 4, 128, 9, 128], BF16, isOutput=False)
    # avec: [bn1s(4) bn1b(4) bn2s(4) bn2b(4)] = [128, 16]
    avec_d = nc.declare_dram_parameter("avec", [128, 16], F32, isOutput=False)
    # hmask: [top-halo-valid, bottom-halo-valid] per rank (0.0 at image edge)
    hmask_d = nc.declare_dram_parameter("hmask", [128, 2], F32, isOutput=False)
    out_d = nc.declare_dram_parameter("out", [128, 4, BW * W], F32, isOutput=True)

    with tile.TileContext(nc) as tc:
        import contextlib

        ctx = contextlib.ExitStack()
        with ctx:
            const = ctx.enter_context(tc.tile_pool(name="const", bufs=1))
            kq = ctx.enter_context(tc.tile_pool(name="kq", bufs=1))
            xtp = ctx.enter_context(tc.tile_pool(name="xtp", bufs=1))
            rcp = ctx.enter_context(tc.tile_pool(name="rcp", bufs=2))
            stgp = ctx.enter_context(tc.tile_pool(name="stgp", bufs=1))
            scps = ctx.enter_context(tc.tile_pool(name="scps", bufs=2, space="PSUM"))
            yps = ctx.enter_context(tc.tile_pool(name="yps", bufs=1, space="PSUM"))
            smps = ctx.enter_context(tc.tile_pool(name="smps", bufs=2, space="PSUM"))
            dram = ctx.enter_context(tc.tile_pool(name="dram", bufs=1, space="DRAM"))

            # ---- activations + weights (DMA priority order) --------------
            x_sb = const.tile([128, 4, S], BF16)
            nc.sync.dma_start(out=x_sb, in_=x4_d[:].rearrange("k p s -> p k s"))
            wq_sb = const.tile([128, 3, 4, 128], BF16)
            wk_sb = const.tile([128, 3, 4, 128], BF16)
            nc.sync.dma_start(out=wk_sb, in_=wkT_d[:].rearrange("i k p f -> p i k f"))
            nc.sync.dma_start(out=wq_sb, in_=wqT_d[:].rearrange("i k p f -> p i k f"))
            othp = ctx.enter_context(tc.tile_pool(name="othp", bufs=1))
            oth1 = othp.tile([128, 4, S], BF16, name="oth")
            nc.sync.dma_start(out=oth1, in_=oth_d[0].rearrange("k p s -> p k s"))
            wvo_sb = const.tile([128, 4, 384], BF16)
            nc.sync.dma_start(out=wvo_sb, in_=wvoT_d[:].rearrange("k p f -> p k f"))

            # Small consts: DMA to staging, then re-own on the consuming
            # engine so consumers need no cross-engine const wait.
            dvec_st = const.tile([128, 6], F32)
            nc.gpsimd.dma_start(out=dvec_st, in_=dvec_d[:])
            wobv_st = const.tile([128, 384], F32)
            nc.gpsimd.dma_start(out=wobv_st, in_=wobv_d[:].partition_broadcast(128))
            avec_st = const.tile([128, 16], F32)
            nc.gpsimd.dma_start(out=avec_st, in_=avec_d[:])
            hmask_st = const.tile([128, 2], F32)
            nc.gpsimd.dma_start(out=hmask_st, in_=hmask_d[:])
            hmask = const.tile([128, 2], F32)
            nc.vector.tensor_copy(hmask, hmask_st)
            dvec = const.tile([128, 6], F32)
            nc.vector.tensor_copy(dvec, dvec_st)
            wobv_sb = const.tile([128, 384], F32)
            nc.vector.tensor_copy(wobv_sb, wobv_st)
            avec = const.tile([128, 16], F32)
            nc.scalar.activation(out=avec, in_=avec_st, func=AF.Copy)
            bqv_sb = dvec[:, 0:3]
            bkv_sb = dvec[:, 3:6]

            # xt (attention output) slabs, zero-padded 34x34
            xt_sl = []
            for i in range(3):
                t = xtp.tile([128, HP, HP], BF16, name=f"xt{i}")
                nc.vector.memset(t, 0.0)
                xt_sl.append(t)

            # Semaphore warmers
            warm = const.tile([128, 1], F32)
            nc.vector.tensor_copy(warm, dvec[:, 0:1])
            warm2 = const.tile([128, 1], F32)
            nc.scalar.activation(out=warm2, in_=warm, func=AF.Copy)

            # k/q per branch (with biases added), uT tiles (ones col at 32)
            k_sb = kq.tile([128, 3, S], BF16)
            q_sb = kq.tile([128, 3, S], BF16)
            uT = [kq.tile([128, 3, 4, 33], BF16, name=f"uT{t}") for t in range(8)]

            # exp outputs: pool of 18 [128, S] bf16 tiles (16 per half live
            # + pipeline slack)
            pt = ctx.enter_context(tc.tile_pool(name="pt", bufs=18))

            qsrc = [oth1, x_sb, None]

            def load_oth2():
                # same pool slot as oth1 (bufs=1): x_prev overwrites x_next
                # once q_proj(0) has consumed it
                oth2 = othp.tile([128, 4, S], BF16, name="oth")
                nc.sync.dma_start(out=oth2, in_=oth_d[1].rearrange("k p s -> p k s"))
                qsrc[2] = oth2

            def k_proj(i):
                for s in range(2):
                    ps = smps.tile([128, 512], F32, name="proj_ps", tag="sm")
                    for ks in range(4):
                        nc.tensor.matmul(
                            ps,
                            lhsT=wk_sb[:, i, ks, :],
                            rhs=x_sb[:, ks, 512 * s : 512 * (s + 1)],
                            start=(ks == 0),
                            stop=(ks == 3),
                        )
                    nc.vector.tensor_scalar_add(
                        k_sb[:, i, 512 * s : 512 * (s + 1)], ps, bkv_sb[:, i : i + 1]
                    )

            def q_proj(i):
                for s in range(2):
                    ps = smps.tile([128, 512], F32, name="proj_ps", tag="sm")
                    for ks in range(4):
                        nc.tensor.matmul(
                            ps,
                            lhsT=wq_sb[:, i, ks, :],
                            rhs=qsrc[i][:, ks, 512 * s : 512 * (s + 1)],
                            start=(ks == 0),
                            stop=(ks == 3),
                        )
                    nc.vector.tensor_scalar_add(
                        q_sb[:, i, 512 * s : 512 * (s + 1)], ps, bqv_sb[:, i : i + 1]
                    )

            def u_proj(t):
                u_ps = smps.tile([128, 512], F32, name="proj_ps", tag="sm")
                for ks in range(4):
                    nc.tensor.matmul(
                        u_ps[:, 0:384],
                        lhsT=x_sb[:, ks, 128 * t : 128 * (t + 1)],
                        rhs=wvo_sb[:, ks, :],
                        start=(ks == 0),
                        stop=(ks == 3),
                    )
                nc.vector.memset(uT[t][:, :, :, 32:33], 1.0)
                nc.vector.tensor_add(
                    uT[t][:, :, :, 0:32],
                    u_ps[:, 0:384].rearrange("p (i h d) -> p i h d", i=3, h=4),
                    wobv_sb.rearrange("p (i h d) -> p i h d", i=3, h=4),
                )

            # ---- conv weights ------------------------------------------
            convw = ctx.enter_context(tc.tile_pool(name="convw", bufs=1))
            c1w_sb = [
                [convw.tile([128, 9, 128], BF16, name=f"c1w{i}_{m}") for m in range(4)]
                for i in range(3)
            ]
            c2w_sb = [
                [convw.tile([128, 9, 128], BF16, name=f"c2w{k}_{m}") for m in range(4)]
                for k in range(4)
            ]

            def load_conv1_w():
                for i in range(3):
                    for m in range(4):
                        nc.sync.dma_start(out=c1w_sb[i][m], in_=c1wT_d[i, m])

            def load_conv2_w():
                for k in range(4):
                    for m in range(4):
                        nc.sync.dma_start(out=c2w_sb[k][m], in_=c2wT_d[k, m])

            # ---- conv1 partial staging + band RS ------------------------
            # staged[m]: [128, 34, 32] bf16; row r+1 = image row r, rows 0/33
            # zero (SAME-pad at image top/bottom => also the RS halo pad).
            staged = [stgp.tile([128, HP, W], BF16, name=f"stg{m}") for m in range(4)]
            for m in range(4):
                nc.vector.memset(staged[m][:, 0:1, :], 0.0)
                nc.vector.memset(staged[m][:, 33:34, :], 0.0)

            rsin = [dram.tile([4, 512, BR, W], BF16, name=f"rsin{i}") for i in range(3)]
            rsout = [dram.tile([512, BR, W], BF16, name=f"rsout{i}") for i in range(3)]

            def conv1_block(i, m, n):
                """Partial conv1 for branch i, out m-tile, spatial half n ->
                staged[m] (bf16)."""
                ps = smps.tile([128, 512], F32, name="cv", tag="sm")
                for dy in range(3):
                    for dx in range(3):
                        nc.tensor.matmul(
                            ps,
                            lhsT=c1w_sb[i][m][:, dy * 3 + dx, :],
                            rhs=xt_sl[i][:, 16 * n + dy : 16 * n + dy + 16, dx : dx + 32],
                            start=(dy == 0 and dx == 0),
                            stop=(dy == 2 and dx == 2),
                        )
                nc.vector.tensor_copy(
                    staged[m][:, 1 + 16 * n : 17 + 16 * n, :],
                    ps.rearrange("p (a b) -> p a b", b=32),
                )
                if n == 1:
                    # ship m-tile into the 4 overlapping band-chunks of rsin[i]
                    for jj in range(4):
                        nc.gpsimd.dma_start(
                            out=rsin[i][jj, 128 * m : 128 * (m + 1), :, :],
                            in_=staged[m][:, 8 * jj : 8 * jj + BR, :],
                        )

            def rs_branch(i):
                nc.gpsimd.collective_compute(
                    "ReduceScatter",
                    mybir.AluOpType.add,
                    replica_groups=GROUPS,
                    ins=[rsin[i][:]],
                    outs=[rsout[i][:]],
                )

            # ---- attention halves ---------------------------------------
            # halves j = 0..5: branch i = j // 2, pair p = j % 2,
            # heads (p, p + 2); y col-groups at rows 0 and 64.
            ptt = {}

            def emit_scores(i, h, t):
                sc = scps.tile([128, S], F32, name="sc", tag="sc")
                p0 = 32 * h
                for s in range(2):
                    nc.tensor.matmul(
                        sc[:, 512 * s : 512 * (s + 1)],
                        lhsT=k_sb[p0 : p0 + 32, i, 128 * t : 128 * (t + 1)],
                        rhs=q_sb[p0 : p0 + 32, i, 512 * s : 512 * (s + 1)],
                        start=True,
                        stop=True,
                        tile_position=(p0, 0),
                    )
                ptile = pt.tile([128, S], BF16, name="ptt")
                nc.scalar.activation(out=ptile, in_=sc, func=AF.Exp, scale=float(ISQD))
                ptt[(i, h, t)] = ptile

            def y_quarter(i, p, y, q):
                """Quarter q (0..3) of the y chains for heads (p, p+2).
                Heads run SEQUENTIALLY into rows 0:33 of the same psum tile
                (on HW a matmul `start` zeroes the whole 2KB psum bank, so a
                second head may only start after the first head's rows are
                copied out -- see y_save): q0/q1 = head p t0-3/t4-7,
                q2/q3 = head p+2."""
                h = p if q < 2 else p + 2
                for t in range(4 * (q % 2), 4 * (q % 2) + 4):
                    for s in range(2):
                        nc.tensor.matmul(
                            y[0:33, 512 * s : 512 * (s + 1)],
                            lhsT=uT[t][:, i, h, :],
                            rhs=ptt[(i, h, t)][:, 512 * s : 512 * (s + 1)],
                            start=(t == 0),
                            stop=(t == 7),
                        )

            def y_save(y, ysb, rc, rr):
                # stage the finished head's y rows (bf16) + denominator row
                # (f32, for the ln) to SBUF before the next head's chain
                # re-starts (and bank-zeroes) the psum banks
                nc.vector.tensor_copy(ysb, y[0:32, :])
                nc.vector.tensor_copy(rc[rr : rr + 1, :], y[32:33, :])

            def y_pass(i, p, rc):
                y = yps.tile([33, S], F32, name="y", tag="y")
                ya = rcp.tile([32, S], BF16, name="ya", bufs=1)
                yb = rcp.tile([32, S], BF16, name="yb", bufs=1)
                y_quarter(i, p, y, 0)
                y_quarter(i, p, y, 1)
                y_save(y, ya, rc, 0)
                y_quarter(i, p, y, 2)
                y_quarter(i, p, y, 3)
                y_save(y, yb, rc, 32)
                return (ya, yb)

            # Two persistent rc buffers (alternating per half).  Rows 0/32
            # hold the two denominators; rows 1-31 are pre-set to 1.0 so the
            # batched [33, S] ln/exp reads only defined data (free-dim size
            # drives ACT cost, the extra partitions are free).
            rc_bufs = [rcp.tile([33, S], F32, name=f"rcb{a}", bufs=1) for a in range(2)]
            for a in range(2):
                nc.vector.memset(rc_bufs[a], 1.0)
            rc_idx = [0]

            def next_rc():
                rc = rc_bufs[rc_idx[0] % 2]
                rc_idx[0] += 1
                return rc

            def recip_pass(rc):
                nc.scalar.activation(out=rc, in_=rc, func=AF.Ln)
                nc.scalar.activation(out=rc, in_=rc, func=AF.Exp, scale=-1.0)
                return rc

            def mul_pass(i, p, yt, rc):
                ya, yb = yt
                hA, hB = p, p + 2
                for (h, ysb, rr) in ((hA, ya, 0), (hB, yb, 32)):
                    src = rc[rr : rr + 1, :]
                    if rr != 0:
                        # partition_broadcast reads garbage from non-zero
                        # base partitions on HW: stage through a base-0 tile
                        rc2 = rcp.tile([1, S], F32, name="rc2")
                        nc.vector.tensor_copy(rc2, src)
                        src = rc2[:]
                    rcb = rcp.tile([32, S], F32, name="rcbb")
                    nc.gpsimd.partition_broadcast(rcb, src)
                    nc.vector.tensor_mul(
                        xt_sl[i][32 * h : 32 * h + 32, 1:33, 1:33],
                        ysb[0:32, :].rearrange("p (a b) -> p a b", b=32),
                        rcb.rearrange("p (a b) -> p a b", b=32),
                    )

            # ---- the pipelined emission ---------------------------------
            # Halves j = 0..5 = (branch j//2, pair j%2).  Slot structure of
            # half j (8 t-slots): each slot emits the 2 heads' score MMs and
            # their exps; the PREVIOUS half's trailing work is injected in
            # quarter-granular pieces so the PE FIFO never blocks scores:
            #   slots 0-3: one y-quarter each; slot 4: recip; slot 5: muls;
            #   slots 2-7: filler thunks (projections / conv1 / RS issues);
            # fillers must be data-ready at their FIFO position (conv1 of
            # branch b only after branch b's muls are emitted).
            HALVES = [(j // 2, j % 2) for j in range(6)]

            c1q = {
                i: [(lambda i=i, m=m, n=n: conv1_block(i, m, n))
                    for m in range(4) for n in range(2)]
                for i in range(3)
            }
            # fillers[j][t] = list of thunks for half j, slot t
            fillers = {j: {t: [] for t in range(8)} for j in range(6)}
            for t in range(4):
                fillers[0][2 + t] = [lambda t=t: u_proj(t)]
            fillers[0][6] = [lambda: u_proj(4), lambda: u_proj(5)]
            fillers[0][7] = [lambda: u_proj(6), lambda: u_proj(7)]
            fillers[1][3] = [load_conv1_w]
            fillers[1][6] = [lambda: k_proj(1)]
            fillers[1][7] = [lambda: q_proj(1)]
            fillers[2][2] = [lambda: k_proj(2)]
            fillers[2][3] = [lambda: q_proj(2)]
            # xt0 complete after half-2 slot-5 muls -> conv1 br0 from slot 6
            fillers[2][6] = c1q[0][0:1]
            fillers[2][7] = c1q[0][1:2]
            for t in range(6):
                fillers[3][2 + t] = c1q[0][2 + t : 3 + t]
            fillers[4][2] = [lambda: rs_branch(0), load_conv2_w]
            fillers[4][6] = c1q[1][0:1]
            fillers[4][7] = c1q[1][1:2]
            for t in range(6):
                fillers[5][2 + t] = c1q[1][2 + t : 3 + t]

            k_proj(0)
            q_proj(0)
            load_oth2()

            prev = None  # (i, p, ytile) trailing from previous half

            for j, (i, p) in enumerate(HALVES):
                hA, hB = p, p + 2
                for t in range(8):
                    emit_scores(i, hA, t)
                    emit_scores(i, hB, t)
                    if prev is not None:
                        pi, pp = prev
                        if t == 0:
                            ycur = yps.tile([33, S], F32, name="y", tag="y")
                            ysave = (
                                rcp.tile([32, S], BF16, name="ya", bufs=1),
                                rcp.tile([32, S], BF16, name="yb", bufs=1),
                            )
                            cur_rc = next_rc()
                            y_quarter(pi, pp, ycur, 0)
                        elif t == 1:
                            y_quarter(pi, pp, ycur, 1)
                            y_save(ycur, ysave[0], cur_rc, 0)
                        elif t == 2:
                            y_quarter(pi, pp, ycur, 2)
                        elif t == 3:
                            y_quarter(pi, pp, ycur, 3)
                            y_save(ycur, ysave[1], cur_rc, 32)
                        elif t == 4:
                            recip_pass(cur_rc)
                        elif t == 5:
                            mul_pass(pi, pp, ysave, cur_rc)
                    for th in fillers[j][t]:
                        th()
                prev = (i, p)

            # ---- tail: last half's y/recip/mul, conv1 br1 rest + br2,
            # RS chain, h1, conv2 ----------------------------------------
            rc = next_rc()
            ylast = y_pass(2, 1, rc)
            recip_pass(rc)
            mul_pass(2, 1, ylast, rc)
            rs_branch(1)
            for m in range(4):
                for n in range(2):
                    conv1_block(2, m, n)
            rs_branch(2)

            # rsout -> SBUF; h1 = relu(bn1 * (p0+p1+p2) + b); conv2 banded
            rsl = [
                stgp.tile([128, 4, BR, W], BF16, name=f"rsl{i}", bufs=1)
                for i in range(3)
            ]
            for i in range(3):
                nc.sync.dma_start(
                    out=rsl[i],
                    in_=rsout[i][:].rearrange("(m p) r c -> p m r c", p=128),
                )
            racc = stgp.tile([128, 4, BR, W], BF16, name="racc", bufs=1)
            nc.vector.tensor_add(racc, rsl[0], rsl[1])
            nc.vector.tensor_add(racc, racc, rsl[2])

            # h1 band tiles [128, BR, 34] bf16 (width-padded)
            h1b = [stgp.tile([128, BR, HP], BF16, name=f"h1b{k}", bufs=1) for k in range(4)]
            for k in range(4):
                nc.vector.memset(h1b[k][:, :, 0:1], 0.0)
                nc.vector.memset(h1b[k][:, :, 33:34], 0.0)
                nc.scalar.activation(
                    out=h1b[k][:, :, 1:33],
                    in_=racc[:, k],
                    func=AF.Relu,
                    bias=avec[:, 4 + k : 5 + k],
                    scale=avec[:, k : k + 1],
                )
                # halo rows beyond the image edge must be exact zeros for
                # conv2's SAME padding (relu(bias) otherwise leaks in)
                nc.vector.tensor_scalar_mul(
                    h1b[k][:, 0:1, 1:33], h1b[k][:, 0:1, 1:33], hmask[:, 0:1]
                )
                nc.vector.tensor_scalar_mul(
                    h1b[k][:, 9:10, 1:33], h1b[k][:, 9:10, 1:33], hmask[:, 1:2]
                )

            oout = stgp.tile([128, 4, BW * W], F32, name="oout", bufs=1)
            for m in range(4):
                ps = smps.tile([128, 512], F32, name="cv2", tag="sm")
                for k in range(4):
                    for dy in range(3):
                        for dx in range(3):
                            nc.tensor.matmul(
                                ps[:, 0 : BW * W],
                                lhsT=c2w_sb[k][m][:, dy * 3 + dx, :],
                                rhs=h1b[k][:, dy : dy + BW, dx : dx + 32],
                                start=(k == 0 and dy == 0 and dx == 0),
                                stop=(k == 3 and dy == 2 and dx == 2),
                            )
                nc.scalar.activation(
                    out=oout[:, m],
                    in_=ps[:, 0 : BW * W],
                    func=AF.Relu,
                    bias=avec[:, 12 + m : 13 + m],
                    scale=avec[:, 8 + m : 9 + m],
                )
                nc.sync.dma_start(out=out_d[:, m], in_=oout[:, m])

    nc.finalize()
    return nc


def _f(x):
    return np.ascontiguousarray(x, dtype=np.float32)


def _bf(x):
    return np.ascontiguousarray(np.asarray(x, dtype=np.float32).astype(ml_dtypes.bfloat16))


def prepare_core_inputs(inp):
    """Build the 8 per-core input dicts from the full-problem inputs."""
    inp = {k: np.asarray(v, dtype=np.float64) for k, v in inp.items()}
    x = inp["x"].reshape(B, C, S)
    xp = inp["x_prev"].reshape(B, C, S)
    xn = inp["x_next"].reshape(B, C, S)

    bn1s_full = inp["bn1g"] / np.sqrt(inp["bn1v"] + EPS)
    bn1b_full = inp["bn1b"] - inp["bn1m"] * bn1s_full
    bn2s_full = inp["bn2g"] / np.sqrt(inp["bn2v"] + EPS)
    bn2b_full = inp["bn2b"] - inp["bn2m"] * bn2s_full

    # conv2 weights: full, same for every rank: c2wT[k][m] = [128 in-part, 9, 128 out]
    c2wT = np.stack(
        [
            np.stack(
                [
                    inp["c2w"][128 * m : 128 * (m + 1), 128 * k : 128 * (k + 1)]
                    .transpose(1, 2, 3, 0)
                    .reshape(128, 9, 128)
                    for m in range(4)
                ]
            )
            for k in range(4)
        ]
    )
    avec = np.concatenate(
        [
            bn1s_full.reshape(4, 128).T,
            bn1b_full.reshape(4, 128).T,
            bn2s_full.reshape(4, 128).T,
            bn2b_full.reshape(4, 128).T,
        ],
        axis=1,
    )  # (128, 16)

    per_g = []
    for g in range(4):
        sl = slice(128 * g, 128 * (g + 1))
        wqT = np.stack(
            [
                np.stack([inp["Wq"][i][sl, 128 * k : 128 * (k + 1)].T for k in range(4)])
                for i in range(3)
            ]
        )
        wkT = np.stack(
            [
                np.stack([inp["Wk"][i][sl, 128 * k : 128 * (k + 1)].T for k in range(4)])
                for i in range(3)
            ]
        )
        bqv = np.stack([inp["bq"][i][sl] for i in range(3)], axis=1)
        bkv = np.stack([inp["bk"][i][sl] for i in range(3)], axis=1)

        att_s = np.stack(
            [inp["bng"][i][sl] / np.sqrt(inp["bnv"][i][sl] + EPS) for i in range(3)]
        )  # (3,128)
        xtb = np.stack(
            [
                inp["bnb"][i][sl] + (inp["bo"][i][sl] - inp["bnm"][i][sl]) * att_s[i]
                for i in range(3)
            ]
        )  # (3,128)

        wvo_rows = []
        wobv_row = []
        for i in range(3):
            for hl in range(4):
                hg = 4 * g + hl
                wv_h = inp["Wv"][i][32 * hg : 32 * (hg + 1), :]  # (32, 512)
                bv_h = inp["bv"][i][32 * hg : 32 * (hg + 1)]
                wo_h = inp["Wo"][i, hg]  # (32, 32)
                sc = att_s[i][32 * hl : 32 * (hl + 1)]  # (32,)
                wvo_rows.append(sc[:, None] * (wo_h @ wv_h))
                wobv_row.append(sc * (wo_h @ bv_h) + xtb[i][32 * hl : 32 * (hl + 1)])
        wvo_all = np.concatenate(wvo_rows, axis=0)  # (384, 512)
        wobv = np.concatenate(wobv_row)[None, :]  # (1, 384)
        wvoT = np.stack([wvo_all[:, 128 * k : 128 * (k + 1)].T for k in range(4)])

        c1wT = np.stack(
            [
                np.stack(
                    [
                        inp["c1w"][
                            128 * m : 128 * (m + 1),
                            512 * i + 128 * g : 512 * i + 128 * (g + 1),
                        ]
                        .transpose(1, 2, 3, 0)
                        .reshape(128, 9, 128)
                        for m in range(4)
                    ]
                )
                for i in range(3)
            ]
        )

        per_g.append(
            dict(
                wqT=_bf(wqT), wkT=_bf(wkT), wvoT=_bf(wvoT),
                wobv=_f(wobv), c1wT=_bf(c1wT), c2wT=_bf(c2wT),
                dvec=_f(np.concatenate([bqv, bkv], axis=1)),
                avec=_f(avec),
            )
        )

    in_maps = []
    for c in range(NCORES):
        b, g = c // 4, c % 4
        d = dict(per_g[g])
        d["x4"] = _bf(x[b].reshape(4, 128, S))
        d["oth"] = _bf(np.stack([xn[b].reshape(4, 128, S), xp[b].reshape(4, 128, S)]))
        hm = np.ones((128, 2), dtype=np.float32)
        if g == 0:
            hm[:, 0] = 0.0
        if g == 3:
            hm[:, 1] = 0.0
        d["hmask"] = hm
        in_maps.append(d)
    return in_maps


_NC_CACHE = {}


def get_nc():
    if "nc" not in _NC_CACHE:
        _NC_CACHE["nc"] = build_nc()
    return _NC_CACHE["nc"]


def assemble(results):
    out = np.zeros((B, C, H, W), dtype=np.float32)
    for c in range(NCORES):
        b, g = c // 4, c % 4
        o = results[c]["out"].reshape(128, 4, BW, W)
        for m in range(4):
            out[b, 128 * m : 128 * (m + 1), BW * g : BW * (g + 1), :] = o[:, m]
    return out


def kernel(**inputs):
    nc = get_nc()
    in_maps = prepare_core_inputs(inputs)
    res = run_bass_kernel_spmd(nc, in_maps, list(range(NCORES)))
    return assemble(res.results)


# revision 37
# speedup vs baseline: 1.1091x; 1.0907x over previous
"""CSAEncoder Trainium2 kernel v3: ACT-saturated attention + band-ReduceScatter convs.

Sharding (8 cores): 2 batch groups x 4 tensor ranks.  Core c: batch b = c // 4,
rank g = c % 4.

Attention (per core): heads [4g, 4g+4) of all 3 branches for batch b.
Processed as 6 "halves" (3 branches x head-pairs {0,2} / {1,3}):
  - exps stream t-major on ACT (the binding engine: 96 x [128,1024] exps
    ~ 110us).  2 rotating score psum tiles keep ACT fed.
  - per-half y-pass at the half boundary: 2-way column-tile-packed matmuls
    (M=33 incl. the ones-row denominator) at PE col-groups 0 and 64, emitted
    with a 2-t-slot lag into the next half so ACT never stalls.
  - reciprocals: DVE copies the two denominator rows to a compact [2,1024]
    tile; ACT does ln + exp(-x) (natural_log_exp_and_others table set);
    gpsimd broadcasts; DVE multiplies into the padded xt slab.

Convs: conv1 partials (contract own 128 ch/branch, full space) are staged
into 4 overlapping row-bands of 10 rows (8-row band + 1-row halo, zero rows
at the image edges) and ReduceScattered per branch: the RS chunk routing
delivers each rank ITS band fully reduced, halo included -- rank-dependent
band selection without breaking SPMD.  conv2 then runs fully local on the
band (all 512 out channels, N=256 matmuls), output is banded: core owns
out[b, :, 8g:8g+8, :].
"""

import os
import sys

import ml_dtypes
import numpy as np

for _p in ("/opt/trn_rl_repo",):
    if _p not in sys.path and os.path.isdir(_p):
        sys.path.insert(0, _p)

import concourse.bass as bass
import concourse.mybir as mybir
import concourse.tile as tile
from concourse import bacc
from concourse.bass_utils import run_bass_kernel_spmd

F32 = mybir.dt.float32
BF16 = mybir.dt.bfloat16
AF = mybir.ActivationFunctionType

# Restrict ACT table matching to the one set containing exp, ln, relu, copy
# so exactly one table load is emitted (see baseline kernel notes).
_ACT_KEEP_SET = "natural_log_exp_and_others"
_orig_get_act_tables = bacc.get_activation_tables


def _patched_get_act_tables(arch):
    tabs = _orig_get_act_tables(arch)
    return {n: (fns if n == _ACT_KEEP_SET else set()) for n, fns in tabs.items()}


bacc.get_activation_tables = _patched_get_act_tables

B, C, H, W, HEADS = 2, 512, 32, 32, 16
D = C // HEADS            # 32
S = H * W                 # 1024
EPS = 1e-5
ISQD = 1.0 / np.sqrt(D)
NCORES = 8
GROUPS = [[0, 1, 2, 3], [4, 5, 6, 7]]
HP = W + 2                # padded row stride for xt slabs (34)
BR = 10                   # band rows incl 1-row halo each side
BW = 8                    # band rows owned


def build_nc():
    nc = bacc.Bacc(None, target_bir_lowering=False)

    # ---- per-core external inputs -------------------------------------
    x4_d = nc.declare_dram_parameter("x4", [4, 128, S], BF16, isOutput=False)
    oth_d = nc.declare_dram_parameter("oth", [2, 4, 128, S], BF16, isOutput=False)
    wqT_d = nc.declare_dram_parameter("wqT", [3, 4, 128, 128], BF16, isOutput=False)
    wkT_d = nc.declare_dram_parameter("wkT", [3, 4, 128, 128], BF16, isOutput=False)
    wvoT_d = nc.declare_dram_parameter("wvoT", [4, 128, 384], BF16, isOutput=False)
    dvec_d = nc.declare_dram_parameter("dvec", [128, 6], F32, isOutput=False)
    wobv_d = nc.declare_dram_parameter("wobv", [1, 384], F32, isOutput=False)
    c1wT_d = nc.declare_dram_parameter("c1wT", [3, 4, 128, 9, 128], BF16, isOutput=False)
    # full conv2 weights: [k-chunk, m-chunk, 128 in-part, 9 taps, 128 out]
    c2wT_d = nc.declare_dram_parameter("c2wT", [4, 4, 128, 9, 128], BF16, isOutput=False)
    # avec: [bn1s(4) bn1b(4) bn2s(4) bn2b(4)] = [128, 16]
    avec_d = nc.declare_dram_parameter("avec", [128, 16], F32, isOutput=False)
    # hmask: [top-halo-valid, bottom-halo-valid] per rank (0.0 at image edge)
    hmask_d = nc.declare_dram_parameter("hmask", [128, 2], F32, isOutput=False)
    out_d = nc.declare_dram_parameter("out", [128, 4, BW * W], F32, isOutput=True)

    with tile.TileContext(nc) as tc:
        import contextlib

        ctx = contextlib.ExitStack()
        with ctx:
            const = ctx.enter_context(tc.tile_pool(name="const", bufs=1))
            kq = ctx.enter_context(tc.tile_pool(name="kq", bufs=1))
            xtp = ctx.enter_context(tc.tile_pool(name="xtp", bufs=1))
            rcp = ctx.enter_context(tc.tile_pool(name="rcp", bufs=2))
            stgp = ctx.enter_context(tc.tile_pool(name="stgp", bufs=1))
            scps = ctx.enter_context(tc.tile_pool(name="scps", bufs=2, space="PSUM"))
            yps = ctx.enter_context(tc.tile_pool(name="yps", bufs=1, space="PSUM"))
            smps = ctx.enter_context(tc.tile_pool(name="smps", bufs=2, space="PSUM"))
            dram = ctx.enter_context(tc.tile_pool(name="dram", bufs=1, space="DRAM"))

            # ---- activations + weights (DMA priority order) --------------
            x_sb = const.tile([128, 4, S], BF16)
            nc.sync.dma_start(out=x_sb, in_=x4_d[:].rearrange("k p s -> p k s"))
            wq_sb = const.tile([128, 3, 4, 128], BF16)
            wk_sb = const.tile([128, 3, 4, 128], BF16)
            nc.sync.dma_start(out=wk_sb, in_=wkT_d[:].rearrange("i k p f -> p i k f"))
            nc.sync.dma_start(out=wq_sb, in_=wqT_d[:].rearrange("i k p f -> p i k f"))
            othp = ctx.enter_context(tc.tile_pool(name="othp", bufs=1))
            oth1 = othp.tile([128, 4, S], BF16, name="oth")
            nc.sync.dma_start(out=oth1, in_=oth_d[0].rearrange("k p s -> p k s"))
            wvo_sb = const.tile([128, 4, 384], BF16)
            nc.sync.dma_start(out=wvo_sb, in_=wvoT_d[:].rearrange("k p f -> p k f"))

            # Small consts: DMA to staging, then re-own on the consuming
            # engine so consumers need no cross-engine const wait.
            dvec_st = const.tile([128, 6], F32)
            nc.gpsimd.dma_start(out=dvec_st, in_=dvec_d[:])
            wobv_st = const.tile([128, 384], F32)
            nc.gpsimd.dma_start(out=wobv_st, in_=wobv_d[:].partition_broadcast(128))
            avec_st = const.tile([128, 16], F32)
            nc.gpsimd.dma_start(out=avec_st, in_=avec_d[:])
            hmask_st = const.tile([128, 2], F32)
            nc.gpsimd.dma_start(out=hmask_st, in_=hmask_d[:])
            hmask = const.tile([128, 2], F32)
            nc.vector.tensor_copy(hmask, hmask_st)
            dvec = const.tile([128, 6], F32)
            nc.vector.tensor_copy(dvec, dvec_st)
            wobv_sb = const.tile([128, 384], F32)
            nc.vector.tensor_copy(wobv_sb, wobv_st)
            avec = const.tile([128, 16], F32)
            nc.scalar.activation(out=avec, in_=avec_st, func=AF.Copy)
            bqv_sb = dvec[:, 0:3]
            bkv_sb = dvec[:, 3:6]

            # xt (attention output) slabs, zero-padded 34x34
            xt_sl = []
            for i in range(3):
                t = xtp.tile([128, HP, HP], BF16, name=f"xt{i}")
                nc.vector.memset(t, 0.0)
                xt_sl.append(t)

            # Semaphore warmers
            warm = const.tile([128, 1], F32)
            nc.vector.tensor_copy(warm, dvec[:, 0:1])
            warm2 = const.tile([128, 1], F32)
            nc.scalar.activation(out=warm2, in_=warm, func=AF.Copy)

            # k/q per branch (with biases added), uT tiles (ones col at 32)
            k_sb = kq.tile([128, 3, S], BF16)
            q_sb = kq.tile([128, 3, S], BF16)
            uT = [kq.tile([128, 3, 4, 33], BF16, name=f"uT{t}") for t in range(8)]

            # exp outputs: pool of 18 [128, S] bf16 tiles (16 per half live
            # + pipeline slack)
            pt = ctx.enter_context(tc.tile_pool(name="pt", bufs=18))

            qsrc = [oth1, x_sb, None]

            def load_oth2():
                # same pool slot as oth1 (bufs=1): x_prev overwrites x_next
                # once q_proj(0) has consumed it
                oth2 = othp.tile([128, 4, S], BF16, name="oth")
                nc.sync.dma_start(out=oth2, in_=oth_d[1].rearrange("k p s -> p k s"))
                qsrc[2] = oth2

            def k_proj(i):
                for s in range(2):
                    ps = smps.tile([128, 512], F32, name="proj_ps", tag="sm")
                    for ks in range(4):
                        nc.tensor.matmul(
                            ps,
                            lhsT=wk_sb[:, i, ks, :],
                            rhs=x_sb[:, ks, 512 * s : 512 * (s + 1)],
                            start=(ks == 0),
                            stop=(ks == 3),
                        )
                    nc.vector.tensor_scalar_add(
                        k_sb[:, i, 512 * s : 512 * (s + 1)], ps, bkv_sb[:, i : i + 1]
                    )

            def q_proj(i):
                for s in range(2):
                    ps = smps.tile([128, 512], F32, name="proj_ps", tag="sm")
                    for ks in range(4):
                        nc.tensor.matmul(
                            ps,
                            lhsT=wq_sb[:, i, ks, :],
                            rhs=qsrc[i][:, ks, 512 * s : 512 * (s + 1)],
                            start=(ks == 0),
                            stop=(ks == 3),
                        )
                    nc.vector.tensor_scalar_add(
                        q_sb[:, i, 512 * s : 512 * (s + 1)], ps, bqv_sb[:, i : i + 1]
                    )

            def u_proj(t):
                u_ps = smps.tile([128, 512], F32, name="proj_ps", tag="sm")
                for ks in range(4):
                    nc.tensor.matmul(
                        u_ps[:, 0:384],
                        lhsT=x_sb[:, ks, 128 * t : 128 * (t + 1)],
                        rhs=wvo_sb[:, ks, :],
                        start=(ks == 0),
                        stop=(ks == 3),
                    )
                nc.vector.memset(uT[t][:, :, :, 32:33], 1.0)
                nc.vector.tensor_add(
                    uT[t][:, :, :, 0:32],
                    u_ps[:, 0:384].rearrange("p (i h d) -> p i h d", i=3, h=4),
                    wobv_sb.rearrange("p (i h d) -> p i h d", i=3, h=4),
                )

            # ---- conv weights ------------------------------------------
            convw = ctx.enter_context(tc.tile_pool(name="convw", bufs=1))
            c1w_sb = [
                [convw.tile([128, 9, 128], BF16, name=f"c1w{i}_{m}") for m in range(4)]
                for i in range(3)
            ]
            c2w_sb = [
                [convw.tile([128, 9, 128], BF16, name=f"c2w{k}_{m}") for m in range(4)]
                for k in range(4)
            ]

            def load_conv1_w():
                for i in range(3):
                    for m in range(4):
                        nc.sync.dma_start(out=c1w_sb[i][m], in_=c1wT_d[i, m])

            def load_conv2_w():
                for k in range(4):
                    for m in range(4):
                        nc.sync.dma_start(out=c2w_sb[k][m], in_=c2wT_d[k, m])

            # ---- conv1 partial staging + band RS ------------------------
            # staged[m]: [128, 34, 32] bf16; row r+1 = image row r, rows 0/33
            # zero (SAME-pad at image top/bottom => also the RS halo pad).
            staged = [stgp.tile([128, HP, W], BF16, name=f"stg{m}") for m in range(4)]
            for m in range(4):
                nc.vector.memset(staged[m][:, 0:1, :], 0.0)
                nc.vector.memset(staged[m][:, 33:34, :], 0.0)

            rsin = [dram.tile([4, 512, BR, W], BF16, name=f"rsin{i}") for i in range(2)]
            rsout = [dram.tile([512, BR, W], BF16, name=f"rsout{i}") for i in range(2)]
            # branch 2: two half-channel RS chunks (pipelined tail)
            rsin2 = [dram.tile([4, 256, BR, W], BF16, name=f"rsin2_{a}") for a in range(2)]
            rsout2 = [dram.tile([256, BR, W], BF16, name=f"rsout2_{a}") for a in range(2)]

            def conv1_block(i, m, n):
                """Partial conv1 for branch i, out m-tile, spatial half n ->
                staged[m] (bf16)."""
                ps = smps.tile([128, 512], F32, name="cv", tag="sm")
                for dy in range(3):
                    for dx in range(3):
                        nc.tensor.matmul(
                            ps,
                            lhsT=c1w_sb[i][m][:, dy * 3 + dx, :],
                            rhs=xt_sl[i][:, 16 * n + dy : 16 * n + dy + 16, dx : dx + 32],
                            start=(dy == 0 and dx == 0),
                            stop=(dy == 2 and dx == 2),
                        )
                nc.vector.tensor_copy(
                    staged[m][:, 1 + 16 * n : 17 + 16 * n, :],
                    ps.rearrange("p (a b) -> p a b", b=32),
                )
                if n == 1:
                    # ship m-tile into the 4 overlapping band-chunks
                    for jj in range(4):
                        if i < 2:
                            dst = rsin[i][jj, 128 * m : 128 * (m + 1), :, :]
                        else:
                            dst = rsin2[m // 2][jj, 128 * (m % 2) : 128 * (m % 2) + 128]
                        nc.gpsimd.dma_start(
                            out=dst, in_=staged[m][:, 8 * jj : 8 * jj + BR, :]
                        )

            def rs_branch(i):
                nc.gpsimd.collective_compute(
                    "ReduceScatter",
                    mybir.AluOpType.add,
                    replica_groups=GROUPS,
                    ins=[rsin[i][:]],
                    outs=[rsout[i][:]],
                )

            def rs2_chunk(a):
                nc.gpsimd.collective_compute(
                    "ReduceScatter",
                    mybir.AluOpType.add,
                    replica_groups=GROUPS,
                    ins=[rsin2[a][:]],
                    outs=[rsout2[a][:]],
                )

            # ---- attention halves ---------------------------------------
            # halves j = 0..5: branch i = j // 2, pair p = j % 2,
            # heads (p, p + 2); y col-groups at rows 0 and 64.
            ptt = {}

            def emit_scores(i, h, t):
                sc = scps.tile([128, S], F32, name="sc", tag="sc")
                p0 = 32 * h
                for s in range(2):
                    nc.tensor.matmul(
                        sc[:, 512 * s : 512 * (s + 1)],
                        lhsT=k_sb[p0 : p0 + 32, i, 128 * t : 128 * (t + 1)],
                        rhs=q_sb[p0 : p0 + 32, i, 512 * s : 512 * (s + 1)],
                        start=True,
                        stop=True,
                        tile_position=(p0, 0),
                    )
                ptile = pt.tile([128, S], BF16, name="ptt")
                nc.scalar.activation(out=ptile, in_=sc, func=AF.Exp, scale=float(ISQD))
                ptt[(i, h, t)] = ptile

            def y_quarter(i, p, y, q):
                """Quarter q (0..3) of the y chains for heads (p, p+2).
                Heads run SEQUENTIALLY into rows 0:33 of the same psum tile
                (on HW a matmul `start` zeroes the whole 2KB psum bank, so a
                second head may only start after the first head's rows are
                copied out -- see y_save): q0/q1 = head p t0-3/t4-7,
                q2/q3 = head p+2."""
                h = p if q < 2 else p + 2
                for t in range(4 * (q % 2), 4 * (q % 2) + 4):
                    for s in range(2):
                        nc.tensor.matmul(
                            y[0:33, 512 * s : 512 * (s + 1)],
                            lhsT=uT[t][:, i, h, :],
                            rhs=ptt[(i, h, t)][:, 512 * s : 512 * (s + 1)],
                            start=(t == 0),
                            stop=(t == 7),
                        )

            def y_save(y, ysb, rc, rr):
                # stage the finished head's y rows (bf16) + denominator row
                # (f32, for the ln) to SBUF before the next head's chain
                # re-starts (and bank-zeroes) the psum banks
                nc.vector.tensor_copy(ysb, y[0:32, :])
                nc.vector.tensor_copy(rc[rr : rr + 1, :], y[32:33, :])

            def y_pass(i, p, rc):
                y = yps.tile([33, S], F32, name="y", tag="y")
                ya = rcp.tile([32, S], BF16, name="ya", bufs=1)
                yb = rcp.tile([32, S], BF16, name="yb", bufs=1)
                y_quarter(i, p, y, 0)
                y_quarter(i, p, y, 1)
                y_save(y, ya, rc, 0)
                y_quarter(i, p, y, 2)
                y_quarter(i, p, y, 3)
                y_save(y, yb, rc, 32)
                return (ya, yb)

            # Two persistent rc buffers (alternating per half).  Rows 0/32
            # hold the two denominators; rows 1-31 are pre-set to 1.0 so the
            # batched [33, S] ln/exp reads only defined data (free-dim size
            # drives ACT cost, the extra partitions are free).
            rc_bufs = [rcp.tile([33, S], F32, name=f"rcb{a}", bufs=1) for a in range(2)]
            for a in range(2):
                nc.vector.memset(rc_bufs[a], 1.0)
            rc_idx = [0]

            def next_rc():
                rc = rc_bufs[rc_idx[0] % 2]
                rc_idx[0] += 1
                return rc

            def recip_pass(rc):
                nc.scalar.activation(out=rc, in_=rc, func=AF.Ln)
                nc.scalar.activation(out=rc, in_=rc, func=AF.Exp, scale=-1.0)
                return rc

            def mul_pass(i, p, yt, rc):
                ya, yb = yt
                hA, hB = p, p + 2
                for (h, ysb, rr) in ((hA, ya, 0), (hB, yb, 32)):
                    src = rc[rr : rr + 1, :]
                    if rr != 0:
                        # partition_broadcast reads garbage from non-zero
                        # base partitions on HW: stage through a base-0 tile
                        rc2 = rcp.tile([1, S], F32, name="rc2")
                        nc.vector.tensor_copy(rc2, src)
                        src = rc2[:]
                    rcb = rcp.tile([32, S], F32, name="rcbb")
                    nc.gpsimd.partition_broadcast(rcb, src)
                    nc.vector.tensor_mul(
                        xt_sl[i][32 * h : 32 * h + 32, 1:33, 1:33],
                        ysb[0:32, :].rearrange("p (a b) -> p a b", b=32),
                        rcb.rearrange("p (a b) -> p a b", b=32),
                    )

            # ---- the pipelined emission ---------------------------------
            # Halves j = 0..5 = (branch j//2, pair j%2).  Slot structure of
            # half j (8 t-slots): each slot emits the 2 heads' score MMs and
            # their exps; the PREVIOUS half's trailing work is injected in
            # quarter-granular pieces so the PE FIFO never blocks scores:
            #   slots 0-3: one y-quarter each; slot 4: recip; slot 5: muls;
            #   slots 2-7: filler thunks (projections / conv1 / RS issues);
            # fillers must be data-ready at their FIFO position (conv1 of
            # branch b only after branch b's muls are emitted).
            HALVES = [(j // 2, j % 2) for j in range(6)]

            c1q = {
                i: [(lambda i=i, m=m, n=n: conv1_block(i, m, n))
                    for m in range(4) for n in range(2)]
                for i in range(3)
            }
            # fillers[j][t] = list of thunks for half j, slot t
            fillers = {j: {t: [] for t in range(8)} for j in range(6)}
            for t in range(4):
                fillers[0][2 + t] = [lambda t=t: u_proj(t)]
            fillers[0][6] = [lambda: u_proj(4), lambda: u_proj(5)]
            fillers[0][7] = [lambda: u_proj(6), lambda: u_proj(7)]
            fillers[1][3] = [load_conv1_w]
            fillers[1][6] = [lambda: k_proj(1)]
            fillers[1][7] = [lambda: q_proj(1)]
            fillers[2][2] = [lambda: k_proj(2)]
            fillers[2][3] = [lambda: q_proj(2)]
            # xt0 complete after half-2 slot-5 muls -> conv1 br0 from slot 6
            fillers[2][6] = c1q[0][0:1]
            fillers[2][7] = c1q[0][1:2]
            for t in range(6):
                fillers[3][2 + t] = c1q[0][2 + t : 3 + t]
            fillers[4][2] = [lambda: rs_branch(0), load_conv2_w]
            fillers[4][6] = c1q[1][0:1]
            fillers[4][7] = c1q[1][1:2]
            for t in range(6):
                fillers[5][2 + t] = c1q[1][2 + t : 3 + t]

            k_proj(0)
            q_proj(0)
            load_oth2()

            prev = None  # (i, p, ytile) trailing from previous half

            for j, (i, p) in enumerate(HALVES):
                hA, hB = p, p + 2
                for t in range(8):
                    emit_scores(i, hA, t)
                    emit_scores(i, hB, t)
                    if prev is not None:
                        pi, pp = prev
                        if t == 0:
                            ycur = yps.tile([33, S], F32, name="y", tag="y")
                            ysave = (
                                rcp.tile([32, S], BF16, name="ya", bufs=1),
                                rcp.tile([32, S], BF16, name="yb", bufs=1),
                            )
                            cur_rc = next_rc()
                            y_quarter(pi, pp, ycur, 0)
                        elif t == 1:
                            y_quarter(pi, pp, ycur, 1)
                            y_save(ycur, ysave[0], cur_rc, 0)
                        elif t == 2:
                            y_quarter(pi, pp, ycur, 2)
                        elif t == 3:
                            y_quarter(pi, pp, ycur, 3)
                            y_save(ycur, ysave[1], cur_rc, 32)
                        elif t == 4:
                            recip_pass(cur_rc)
                        elif t == 5:
                            mul_pass(pi, pp, ysave, cur_rc)
                    for th in fillers[j][t]:
                        th()
                prev = (i, p)

            # ---- tail --------------------------------------------------
            # last half's y/recip/mul -> conv1-br2 per m-pair with RS2
            # chunks pipelined -> h1 per k-chunk -> conv2 k-major with SBUF
            # accumulation (each RS chunk unlocks PE work immediately).
            rsl = [
                stgp.tile([128, 4, BR, W], BF16, name=f"rsl{i}", bufs=1)
                for i in range(2)
            ]
            racc = stgp.tile([128, 4, BR, W], BF16, name="racc", bufs=1)

            def preload_rs01():
                for i in range(2):
                    nc.sync.dma_start(
                        out=rsl[i],
                        in_=rsout[i][:].rearrange("(m p) r c -> p m r c", p=128),
                    )

            def add_rs01():
                nc.vector.tensor_add(racc, rsl[0], rsl[1])

            rs_branch(1)
            rc = next_rc()
            ylast = y_pass(2, 1, rc)
            recip_pass(rc)
            mul_pass(2, 1, ylast, rc)
            preload_rs01()
            # conv1-br2: m-tile order with ships ASAP; RS2 chunk a after its
            # two m-tiles shipped
            for m in range(4):
                conv1_block(2, m, 0)
                conv1_block(2, m, 1)
                if m == 1:
                    rs2_chunk(0)
            rs2_chunk(1)
            add_rs01()

            rsl2 = stgp.tile([128, 4, BR, W], BF16, name="rsl2", bufs=1)

            h1b = [stgp.tile([128, BR, HP], BF16, name=f"h1b{k}", bufs=1) for k in range(4)]
            for k in range(4):
                nc.vector.memset(h1b[k][:, :, 0:1], 0.0)
                nc.vector.memset(h1b[k][:, :, 33:34], 0.0)

            def h1_chunk(k):
                """h1b[k] = relu(bn1 * (racc[k] + rsl2[k]) + b) with edge
                halo masking."""
                nc.sync.dma_start(
                    out=rsl2[:, k],
                    in_=rsout2[k // 2][128 * (k % 2) : 128 * (k % 2) + 128],
                )
                nc.vector.tensor_add(rsl2[:, k], rsl2[:, k], racc[:, k])
                nc.scalar.activation(
                    out=h1b[k][:, :, 1:33],
                    in_=rsl2[:, k],
                    func=AF.Relu,
                    bias=avec[:, 4 + k : 5 + k],
                    scale=avec[:, k : k + 1],
                )
                # halo rows beyond the image edge must be exact zeros for
                # conv2's SAME padding (relu(bias) otherwise leaks in)
                nc.vector.tensor_scalar_mul(
                    h1b[k][:, 0:1, 1:33], h1b[k][:, 0:1, 1:33], hmask[:, 0:1]
                )
                nc.vector.tensor_scalar_mul(
                    h1b[k][:, 9:10, 1:33], h1b[k][:, 9:10, 1:33], hmask[:, 1:2]
                )

            # conv2 k-major: per (k, m) a 9-MM chain -> DVE-accumulate into
            # oacc; k01 runs while RS2 chunk 1 is in flight
            oacc = stgp.tile([128, 4, BW * W], F32, name="oacc", bufs=1)

            def conv2_k(k):
                h1_chunk(k)
                for m in range(4):
                    ps = smps.tile([128, 512], F32, name="cv2", tag="sm")
                    for dy in range(3):
                        for dx in range(3):
                            nc.tensor.matmul(
                                ps[:, 0 : BW * W],
                                lhsT=c2w_sb[k][m][:, dy * 3 + dx, :],
                                rhs=h1b[k][:, dy : dy + BW, dx : dx + 32],
                                start=(dy == 0 and dx == 0),
                                stop=(dy == 2 and dx == 2),
                            )
                    if k == 0:
                        nc.vector.tensor_copy(oacc[:, m], ps[:, 0 : BW * W])
                    else:
                        nc.vector.tensor_add(
                            oacc[:, m], oacc[:, m], ps[:, 0 : BW * W]
                        )

            for k in range(4):
                conv2_k(k)
            for m in range(4):
                nc.scalar.activation(
                    out=oacc[:, m],
                    in_=oacc[:, m],
                    func=AF.Relu,
                    bias=avec[:, 12 + m : 13 + m],
                    scale=avec[:, 8 + m : 9 + m],
                )
                nc.sync.dma_start(out=out_d[:, m], in_=oacc[:, m])

    nc.finalize()
    return nc


def _f(x):
    return np.ascontiguousarray(x, dtype=np.float32)


def _bf(x):
    return np.ascontiguousarray(np.asarray(x, dtype=np.float32).astype(ml_dtypes.bfloat16))


def prepare_core_inputs(inp):
    """Build the 8 per-core input dicts from the full-problem inputs."""
    inp = {k: np.asarray(v, dtype=np.float64) for k, v in inp.items()}
    x = inp["x"].reshape(B, C, S)
    xp = inp["x_prev"].reshape(B, C, S)
    xn = inp["x_next"].reshape(B, C, S)

    bn1s_full = inp["bn1g"] / np.sqrt(inp["bn1v"] + EPS)
    bn1b_full = inp["bn1b"] - inp["bn1m"] * bn1s_full
    bn2s_full = inp["bn2g"] / np.sqrt(inp["bn2v"] + EPS)
    bn2b_full = inp["bn2b"] - inp["bn2m"] * bn2s_full

    # conv2 weights: full, same for every rank: c2wT[k][m] = [128 in-part, 9, 128 out]
    c2wT = np.stack(
        [
            np.stack(
                [
                    inp["c2w"][128 * m : 128 * (m + 1), 128 * k : 128 * (k + 1)]
                    .transpose(1, 2, 3, 0)
                    .reshape(128, 9, 128)
                    for m in range(4)
                ]
            )
            for k in range(4)
        ]
    )
    avec = np.concatenate(
        [
            bn1s_full.reshape(4, 128).T,
            bn1b_full.reshape(4, 128).T,
            bn2s_full.reshape(4, 128).T,
            bn2b_full.reshape(4, 128).T,
        ],
        axis=1,
    )  # (128, 16)

    per_g = []
    for g in range(4):
        sl = slice(128 * g, 128 * (g + 1))
        wqT = np.stack(
            [
                np.stack([inp["Wq"][i][sl, 128 * k : 128 * (k + 1)].T for k in range(4)])
                for i in range(3)
            ]
        )
        wkT = np.stack(
            [
                np.stack([inp["Wk"][i][sl, 128 * k : 128 * (k + 1)].T for k in range(4)])
                for i in range(3)
            ]
        )
        bqv = np.stack([inp["bq"][i][sl] for i in range(3)], axis=1)
        bkv = np.stack([inp["bk"][i][sl] for i in range(3)], axis=1)

        att_s = np.stack(
            [inp["bng"][i][sl] / np.sqrt(inp["bnv"][i][sl] + EPS) for i in range(3)]
        )  # (3,128)
        xtb = np.stack(
            [
                inp["bnb"][i][sl] + (inp["bo"][i][sl] - inp["bnm"][i][sl]) * att_s[i]
                for i in range(3)
            ]
        )  # (3,128)

        wvo_rows = []
        wobv_row = []
        for i in range(3):
            for hl in range(4):
                hg = 4 * g + hl
                wv_h = inp["Wv"][i][32 * hg : 32 * (hg + 1), :]  # (32, 512)
                bv_h = inp["bv"][i][32 * hg : 32 * (hg + 1)]
                wo_h = inp["Wo"][i, hg]  # (32, 32)
                sc = att_s[i][32 * hl : 32 * (hl + 1)]  # (32,)
                wvo_rows.append(sc[:, None] * (wo_h @ wv_h))
                wobv_row.append(sc * (wo_h @ bv_h) + xtb[i][32 * hl : 32 * (hl + 1)])
        wvo_all = np.concatenate(wvo_rows, axis=0)  # (384, 512)
        wobv = np.concatenate(wobv_row)[None, :]  # (1, 384)
        wvoT = np.stack([wvo_all[:, 128 * k : 128 * (k + 1)].T for k in range(4)])

        c1wT = np.stack(
            [
                np.stack(
                    [
                        inp["c1w"][
                            128 * m : 128 * (m + 1),
                            512 * i + 128 * g : 512 * i + 128 * (g + 1),
                        ]
                        .transpose(1, 2, 3, 0)
                        .reshape(128, 9, 128)
                        for m in range(4)
                    ]
                )
                for i in range(3)
            ]
        )

        per_g.append(
            dict(
                wqT=_bf(wqT), wkT=_bf(wkT), wvoT=_bf(wvoT),
                wobv=_f(wobv), c1wT=_bf(c1wT), c2wT=_bf(c2wT),
                dvec=_f(np.concatenate([bqv, bkv], axis=1)),
                avec=_f(avec),
            )
        )

    in_maps = []
    for c in range(NCORES):
        b, g = c // 4, c % 4
        d = dict(per_g[g])
        d["x4"] = _bf(x[b].reshape(4, 128, S))
        d["oth"] = _bf(np.stack([xn[b].reshape(4, 128, S), xp[b].reshape(4, 128, S)]))
        hm = np.ones((128, 2), dtype=np.float32)
        if g == 0:
            hm[:, 0] = 0.0
        if g == 3:
            hm[:, 1] = 0.0
        d["hmask"] = hm
        in_maps.append(d)
    return in_maps


_NC_CACHE = {}


def get_nc():
    if "nc" not in _NC_CACHE:
        _NC_CACHE["nc"] = build_nc()
    return _NC_CACHE["nc"]


def assemble(results):
    out = np.zeros((B, C, H, W), dtype=np.float32)
    for c in range(NCORES):
        b, g = c // 4, c % 4
        o = results[c]["out"].reshape(128, 4, BW, W)
        for m in range(4):
            out[b, 128 * m : 128 * (m + 1), BW * g : BW * (g + 1), :] = o[:, m]
    return out


def kernel(**inputs):
    nc = get_nc()
    in_maps = prepare_core_inputs(inputs)
    res = run_bass_kernel_spmd(nc, in_maps, list(range(NCORES)))
    return assemble(res.results)


# revision 39
# speedup vs baseline: 1.1913x; 1.0741x over previous
"""CSAEncoder Trainium2 kernel v3: ACT-saturated attention + band-ReduceScatter convs.

Sharding (8 cores): 2 batch groups x 4 tensor ranks.  Core c: batch b = c // 4,
rank g = c % 4.

Attention (per core): heads [4g, 4g+4) of all 3 branches for batch b.
Processed as 6 "halves" (3 branches x head-pairs {0,2} / {1,3}):
  - exps stream t-major on ACT (the binding engine: 96 x [128,1024] exps
    ~ 110us).  2 rotating score psum tiles keep ACT fed.
  - per-half y-pass at the half boundary: 2-way column-tile-packed matmuls
    (M=33 incl. the ones-row denominator) at PE col-groups 0 and 64, emitted
    with a 2-t-slot lag into the next half so ACT never stalls.
  - reciprocals: DVE copies the two denominator rows to a compact [2,1024]
    tile; ACT does ln + exp(-x) (natural_log_exp_and_others table set);
    gpsimd broadcasts; DVE multiplies into the padded xt slab.

Convs: conv1 partials (contract own 128 ch/branch, full space) are staged
into 4 overlapping row-bands of 10 rows (8-row band + 1-row halo, zero rows
at the image edges) and ReduceScattered per branch: the RS chunk routing
delivers each rank ITS band fully reduced, halo included -- rank-dependent
band selection without breaking SPMD.  conv2 then runs fully local on the
band (all 512 out channels, N=256 matmuls), output is banded: core owns
out[b, :, 8g:8g+8, :].
"""

import os
import sys

import ml_dtypes
import numpy as np

for _p in ("/opt/trn_rl_repo",):
    if _p not in sys.path and os.path.isdir(_p):
        sys.path.insert(0, _p)

import concourse.bass as bass
import concourse.mybir as mybir
import concourse.tile as tile
from concourse import bacc
from concourse.bass_utils import run_bass_kernel_spmd

F32 = mybir.dt.float32
BF16 = mybir.dt.bfloat16
AF = mybir.ActivationFunctionType

# Restrict ACT table matching to the one set containing exp, ln, relu, copy
# so exactly one table load is emitted (see baseline kernel notes).
_ACT_KEEP_SET = "natural_log_exp_and_others"
_orig_get_act_tables = bacc.get_activation_tables


def _patched_get_act_tables(arch):
    tabs = _orig_get_act_tables(arch)
    return {n: (fns if n == _ACT_KEEP_SET else set()) for n, fns in tabs.items()}


bacc.get_activation_tables = _patched_get_act_tables

B, C, H, W, HEADS = 2, 512, 32, 32, 16
D = C // HEADS            # 32
S = H * W                 # 1024
EPS = 1e-5
ISQD = 1.0 / np.sqrt(D)
NCORES = 8
GROUPS = [[0, 1, 2, 3], [4, 5, 6, 7]]
HP = W + 2                # padded row stride for xt slabs (34)
BR = 10                   # band rows incl 1-row halo each side
BW = 8                    # band rows owned


def build_nc():
    nc = bacc.Bacc(None, target_bir_lowering=False)

    # ---- per-core external inputs -------------------------------------
    x4_d = nc.declare_dram_parameter("x4", [4, 128, S], BF16, isOutput=False)
    oth_d = nc.declare_dram_parameter("oth", [2, 4, 128, S], BF16, isOutput=False)
    wqT_d = nc.declare_dram_parameter("wqT", [3, 4, 128, 128], BF16, isOutput=False)
    wkT_d = nc.declare_dram_parameter("wkT", [3, 4, 128, 128], BF16, isOutput=False)
    wvoT_d = nc.declare_dram_parameter("wvoT", [4, 128, 384], BF16, isOutput=False)
    dvec_d = nc.declare_dram_parameter("dvec", [128, 6], F32, isOutput=False)
    wobv_d = nc.declare_dram_parameter("wobv", [1, 384], F32, isOutput=False)
    c1wT_d = nc.declare_dram_parameter("c1wT", [3, 4, 128, 9, 128], BF16, isOutput=False)
    # full conv2 weights: [k-chunk, m-chunk, 128 in-part, 9 taps, 128 out]
    c2wT_d = nc.declare_dram_parameter("c2wT", [4, 4, 128, 9, 128], BF16, isOutput=False)
    # avec: [bn1s(4) bn1b(4) bn2s(4) bn2b(4)] = [128, 16]
    avec_d = nc.declare_dram_parameter("avec", [128, 16], F32, isOutput=False)
    # hmask: [top-halo-valid, bottom-halo-valid] per rank (0.0 at image edge)
    hmask_d = nc.declare_dram_parameter("hmask", [128, 2], F32, isOutput=False)
    out_d = nc.declare_dram_parameter("out", [128, 4, BW * W], F32, isOutput=True)

    with tile.TileContext(nc) as tc:
        import contextlib

        ctx = contextlib.ExitStack()
        with ctx:
            const = ctx.enter_context(tc.tile_pool(name="const", bufs=1))
            kq = ctx.enter_context(tc.tile_pool(name="kq", bufs=1))
            xtp = ctx.enter_context(tc.tile_pool(name="xtp", bufs=1))
            rcp = ctx.enter_context(tc.tile_pool(name="rcp", bufs=2))
            stgp = ctx.enter_context(tc.tile_pool(name="stgp", bufs=1))
            scps = ctx.enter_context(tc.tile_pool(name="scps", bufs=2, space="PSUM"))
            yps = ctx.enter_context(tc.tile_pool(name="yps", bufs=1, space="PSUM"))
            smps = ctx.enter_context(tc.tile_pool(name="smps", bufs=2, space="PSUM"))
            dram = ctx.enter_context(tc.tile_pool(name="dram", bufs=1, space="DRAM"))

            # ---- activations + weights (DMA priority order) --------------
            x_sb = const.tile([128, 4, S], BF16)
            nc.sync.dma_start(out=x_sb, in_=x4_d[:].rearrange("k p s -> p k s"))
            wq_sb = const.tile([128, 3, 4, 128], BF16)
            wk_sb = const.tile([128, 3, 4, 128], BF16)
            nc.sync.dma_start(out=wk_sb, in_=wkT_d[:].rearrange("i k p f -> p i k f"))
            nc.sync.dma_start(out=wq_sb, in_=wqT_d[:].rearrange("i k p f -> p i k f"))
            othp = ctx.enter_context(tc.tile_pool(name="othp", bufs=1))
            oth1 = othp.tile([128, 4, S], BF16, name="oth")
            nc.sync.dma_start(out=oth1, in_=oth_d[0].rearrange("k p s -> p k s"))
            wvo_sb = const.tile([128, 4, 384], BF16)
            nc.sync.dma_start(out=wvo_sb, in_=wvoT_d[:].rearrange("k p f -> p k f"))

            # Small consts: DMA to staging, then re-own on the consuming
            # engine so consumers need no cross-engine const wait.
            dvec_st = const.tile([128, 6], F32)
            nc.gpsimd.dma_start(out=dvec_st, in_=dvec_d[:])
            wobv_st = const.tile([128, 384], F32)
            nc.gpsimd.dma_start(out=wobv_st, in_=wobv_d[:].partition_broadcast(128))
            avec_st = const.tile([128, 16], F32)
            nc.gpsimd.dma_start(out=avec_st, in_=avec_d[:])
            hmask_st = const.tile([128, 2], F32)
            nc.gpsimd.dma_start(out=hmask_st, in_=hmask_d[:])
            hmask = const.tile([128, 2], F32)
            nc.vector.tensor_copy(hmask, hmask_st)
            dvec = const.tile([128, 6], F32)
            nc.vector.tensor_copy(dvec, dvec_st)
            wobv_sb = const.tile([128, 384], F32)
            nc.vector.tensor_copy(wobv_sb, wobv_st)
            avec = const.tile([128, 16], F32)
            nc.scalar.activation(out=avec, in_=avec_st, func=AF.Copy)
            bqv_sb = dvec[:, 0:3]
            bkv_sb = dvec[:, 3:6]

            # xt (attention output) slabs, zero-padded 34x34
            xt_sl = []
            for i in range(3):
                t = xtp.tile([128, HP, HP], BF16, name=f"xt{i}")
                nc.vector.memset(t, 0.0)
                xt_sl.append(t)

            # Semaphore warmers
            warm = const.tile([128, 1], F32)
            nc.vector.tensor_copy(warm, dvec[:, 0:1])
            warm2 = const.tile([128, 1], F32)
            nc.scalar.activation(out=warm2, in_=warm, func=AF.Copy)

            # k/q per branch (with biases added), uT tiles (ones col at 32)
            k_sb = kq.tile([128, 3, S], BF16)
            q_sb = kq.tile([128, 3, S], BF16)
            uT = [kq.tile([128, 3, 4, 33], BF16, name=f"uT{t}") for t in range(8)]

            # exp outputs: pool of 18 [128, S] bf16 tiles (16 per half live
            # + pipeline slack)
            pt = ctx.enter_context(tc.tile_pool(name="pt", bufs=18))

            qsrc = [oth1, x_sb, None]

            def load_oth2():
                # same pool slot as oth1 (bufs=1): x_prev overwrites x_next
                # once q_proj(0) has consumed it
                oth2 = othp.tile([128, 4, S], BF16, name="oth")
                nc.sync.dma_start(out=oth2, in_=oth_d[1].rearrange("k p s -> p k s"))
                qsrc[2] = oth2

            def k_proj(i):
                for s in range(2):
                    ps = smps.tile([128, 512], F32, name="proj_ps", tag="sm")
                    for ks in range(4):
                        nc.tensor.matmul(
                            ps,
                            lhsT=wk_sb[:, i, ks, :],
                            rhs=x_sb[:, ks, 512 * s : 512 * (s + 1)],
                            start=(ks == 0),
                            stop=(ks == 3),
                        )
                    nc.vector.tensor_scalar_add(
                        k_sb[:, i, 512 * s : 512 * (s + 1)], ps, bkv_sb[:, i : i + 1]
                    )

            def q_proj(i):
                for s in range(2):
                    ps = smps.tile([128, 512], F32, name="proj_ps", tag="sm")
                    for ks in range(4):
                        nc.tensor.matmul(
                            ps,
                            lhsT=wq_sb[:, i, ks, :],
                            rhs=qsrc[i][:, ks, 512 * s : 512 * (s + 1)],
                            start=(ks == 0),
                            stop=(ks == 3),
                        )
                    nc.vector.tensor_scalar_add(
                        q_sb[:, i, 512 * s : 512 * (s + 1)], ps, bqv_sb[:, i : i + 1]
                    )

            def u_proj(t):
                u_ps = smps.tile([128, 512], F32, name="proj_ps", tag="sm")
                for ks in range(4):
                    nc.tensor.matmul(
                        u_ps[:, 0:384],
                        lhsT=x_sb[:, ks, 128 * t : 128 * (t + 1)],
                        rhs=wvo_sb[:, ks, :],
                        start=(ks == 0),
                        stop=(ks == 3),
                    )
                nc.vector.memset(uT[t][:, :, :, 32:33], 1.0)
                nc.vector.tensor_add(
                    uT[t][:, :, :, 0:32],
                    u_ps[:, 0:384].rearrange("p (i h d) -> p i h d", i=3, h=4),
                    wobv_sb.rearrange("p (i h d) -> p i h d", i=3, h=4),
                )

            # ---- conv weights ------------------------------------------
            convw = ctx.enter_context(tc.tile_pool(name="convw", bufs=1))
            c1w_sb = [
                [convw.tile([128, 9, 128], BF16, name=f"c1w{i}_{m}") for m in range(4)]
                for i in range(3)
            ]
            c2w_sb = [
                [convw.tile([128, 9, 128], BF16, name=f"c2w{k}_{m}") for m in range(4)]
                for k in range(4)
            ]

            def load_conv1_w():
                for i in range(3):
                    for m in range(4):
                        nc.sync.dma_start(out=c1w_sb[i][m], in_=c1wT_d[i, m])

            def load_conv2_w():
                for k in range(4):
                    for m in range(4):
                        nc.sync.dma_start(out=c2w_sb[k][m], in_=c2wT_d[k, m])

            # ---- conv1 partial staging + band RS ------------------------
            # staged[m]: [128, 34, 32] bf16; row r+1 = image row r, rows 0/33
            # zero (SAME-pad at image top/bottom => also the RS halo pad).
            staged = [stgp.tile([128, HP, W], BF16, name=f"stg{m}") for m in range(4)]
            for m in range(4):
                nc.vector.memset(staged[m][:, 0:1, :], 0.0)
                nc.vector.memset(staged[m][:, 33:34, :], 0.0)

            rsin = [dram.tile([4, 512, BR, W], BF16, name=f"rsin{i}") for i in range(2)]
            rsout = [dram.tile([512, BR, W], BF16, name=f"rsout{i}") for i in range(2)]
            # branch 2: two half-channel RS chunks (pipelined tail)
            rsin2 = [dram.tile([4, 256, BR, W], BF16, name=f"rsin2_{a}") for a in range(2)]
            rsout2 = [dram.tile([256, BR, W], BF16, name=f"rsout2_{a}") for a in range(2)]

            def conv1_block(i, m, n):
                """Partial conv1 for branch i, out m-tile, spatial half n ->
                staged[m] (bf16)."""
                ps = smps.tile([128, 512], F32, name="cv", tag="sm")
                for dy in range(3):
                    for dx in range(3):
                        nc.tensor.matmul(
                            ps,
                            lhsT=c1w_sb[i][m][:, dy * 3 + dx, :],
                            rhs=xt_sl[i][:, 16 * n + dy : 16 * n + dy + 16, dx : dx + 32],
                            start=(dy == 0 and dx == 0),
                            stop=(dy == 2 and dx == 2),
                        )
                nc.vector.tensor_copy(
                    staged[m][:, 1 + 16 * n : 17 + 16 * n, :],
                    ps.rearrange("p (a b) -> p a b", b=32),
                )
                if n == 1:
                    # ship m-tile into the 4 overlapping band-chunks
                    for jj in range(4):
                        if i < 2:
                            dst = rsin[i][jj, 128 * m : 128 * (m + 1), :, :]
                        else:
                            dst = rsin2[m // 2][jj, 128 * (m % 2) : 128 * (m % 2) + 128]
                        nc.gpsimd.dma_start(
                            out=dst, in_=staged[m][:, 8 * jj : 8 * jj + BR, :]
                        )

            def rs_branch(i):
                nc.gpsimd.collective_compute(
                    "ReduceScatter",
                    mybir.AluOpType.add,
                    replica_groups=GROUPS,
                    ins=[rsin[i][:]],
                    outs=[rsout[i][:]],
                )

            def rs2_chunk(a):
                nc.gpsimd.collective_compute(
                    "ReduceScatter",
                    mybir.AluOpType.add,
                    replica_groups=GROUPS,
                    ins=[rsin2[a][:]],
                    outs=[rsout2[a][:]],
                )

            # ---- attention halves ---------------------------------------
            # halves j = 0..5: branch i = j // 2, pair p = j % 2,
            # heads (p, p + 2); y col-groups at rows 0 and 64.
            ptt = {}

            def emit_scores(i, h, t):
                sc = scps.tile([128, S], F32, name="sc", tag="sc")
                p0 = 32 * h
                for s in range(2):
                    nc.tensor.matmul(
                        sc[:, 512 * s : 512 * (s + 1)],
                        lhsT=k_sb[p0 : p0 + 32, i, 128 * t : 128 * (t + 1)],
                        rhs=q_sb[p0 : p0 + 32, i, 512 * s : 512 * (s + 1)],
                        start=True,
                        stop=True,
                        tile_position=(p0, 0),
                    )
                ptile = pt.tile([128, S], BF16, name="ptt")
                nc.scalar.activation(out=ptile, in_=sc, func=AF.Exp, scale=float(ISQD))
                ptt[(i, h, t)] = ptile

            def y_quarter(i, p, y, q):
                """Quarter q (0..3) of the y chains for heads (p, p+2).
                Heads run SEQUENTIALLY into rows 0:33 of the same psum tile
                (on HW a matmul `start` zeroes the whole 2KB psum bank, so a
                second head may only start after the first head's rows are
                copied out -- see y_save): q0/q1 = head p t0-3/t4-7,
                q2/q3 = head p+2."""
                h = p if q < 2 else p + 2
                for t in range(4 * (q % 2), 4 * (q % 2) + 4):
                    for s in range(2):
                        nc.tensor.matmul(
                            y[0:33, 512 * s : 512 * (s + 1)],
                            lhsT=uT[t][:, i, h, :],
                            rhs=ptt[(i, h, t)][:, 512 * s : 512 * (s + 1)],
                            start=(t == 0),
                            stop=(t == 7),
                        )

            def y_save(y, ysb, rc, rr):
                # stage the finished head's y rows (bf16) + denominator row
                # (f32, for the ln) to SBUF before the next head's chain
                # re-starts (and bank-zeroes) the psum banks
                nc.vector.tensor_copy(ysb, y[0:32, :])
                nc.vector.tensor_copy(rc[rr : rr + 1, :], y[32:33, :])

            def y_pass(i, p, rc):
                y = yps.tile([33, S], F32, name="y", tag="y")
                ya = rcp.tile([32, S], BF16, name="ya", bufs=1)
                yb = rcp.tile([32, S], BF16, name="yb", bufs=1)
                y_quarter(i, p, y, 0)
                y_quarter(i, p, y, 1)
                y_save(y, ya, rc, 0)
                y_quarter(i, p, y, 2)
                y_quarter(i, p, y, 3)
                y_save(y, yb, rc, 32)
                return (ya, yb)

            # Two persistent rc buffers (alternating per half).  Rows 0/32
            # hold the two denominators; rows 1-31 are pre-set to 1.0 so the
            # batched [33, S] ln/exp reads only defined data (free-dim size
            # drives ACT cost, the extra partitions are free).
            rc_bufs = [rcp.tile([33, S], F32, name=f"rcb{a}", bufs=1) for a in range(2)]
            for a in range(2):
                nc.vector.memset(rc_bufs[a], 1.0)
            rc_idx = [0]

            def next_rc():
                rc = rc_bufs[rc_idx[0] % 2]
                rc_idx[0] += 1
                return rc

            def recip_pass(rc):
                nc.scalar.activation(out=rc, in_=rc, func=AF.Ln)
                nc.scalar.activation(out=rc, in_=rc, func=AF.Exp, scale=-1.0)
                return rc

            def mul_pass(i, p, yt, rc):
                ya, yb = yt
                hA, hB = p, p + 2
                for (h, ysb, rr) in ((hA, ya, 0), (hB, yb, 32)):
                    src = rc[rr : rr + 1, :]
                    if rr != 0:
                        # partition_broadcast reads garbage from non-zero
                        # base partitions on HW: stage through a base-0 tile
                        rc2 = rcp.tile([1, S], F32, name="rc2")
                        nc.vector.tensor_copy(rc2, src)
                        src = rc2[:]
                    rcb = rcp.tile([32, S], F32, name="rcbb")
                    nc.gpsimd.partition_broadcast(rcb, src)
                    nc.vector.tensor_mul(
                        xt_sl[i][32 * h : 32 * h + 32, 1:33, 1:33],
                        ysb[0:32, :].rearrange("p (a b) -> p a b", b=32),
                        rcb.rearrange("p (a b) -> p a b", b=32),
                    )

            # ---- the pipelined emission ---------------------------------
            # Halves j = 0..5 = (branch j//2, pair j%2).  Slot structure of
            # half j (8 t-slots): each slot emits the 2 heads' score MMs and
            # their exps; the PREVIOUS half's trailing work is injected in
            # quarter-granular pieces so the PE FIFO never blocks scores:
            #   slots 0-3: one y-quarter each; slot 4: recip; slot 5: muls;
            #   slots 2-7: filler thunks (projections / conv1 / RS issues);
            # fillers must be data-ready at their FIFO position (conv1 of
            # branch b only after branch b's muls are emitted).
            HALVES = [(j // 2, j % 2) for j in range(6)]

            c1q = {
                i: [(lambda i=i, m=m, n=n: conv1_block(i, m, n))
                    for m in range(4) for n in range(2)]
                for i in range(3)
            }
            # fillers[j][t] = list of thunks for half j, slot t
            fillers = {j: {t: [] for t in range(8)} for j in range(6)}
            for t in range(4):
                fillers[0][2 + t] = [lambda t=t: u_proj(t)]
            fillers[0][6] = [lambda: u_proj(4), lambda: u_proj(5)]
            fillers[0][7] = [lambda: u_proj(6), lambda: u_proj(7)]
            fillers[1][3] = [load_conv1_w]
            fillers[1][6] = [lambda: k_proj(1)]
            fillers[1][7] = [lambda: q_proj(1)]
            fillers[2][2] = [lambda: k_proj(2)]
            fillers[2][3] = [lambda: q_proj(2)]
            # xt0 complete after half-2 slot-5 muls -> conv1 br0 from slot 6
            fillers[2][6] = c1q[0][0:1]
            fillers[2][7] = c1q[0][1:2]
            for t in range(6):
                fillers[3][2 + t] = c1q[0][2 + t : 3 + t]
            fillers[4][2] = [lambda: rs_branch(0), load_conv2_w]
            fillers[4][6] = c1q[1][0:1]
            fillers[4][7] = c1q[1][1:2]
            for t in range(6):
                fillers[5][2 + t] = c1q[1][2 + t : 3 + t]

            k_proj(0)
            q_proj(0)
            load_oth2()

            prev = None  # (i, p, ytile) trailing from previous half

            for j, (i, p) in enumerate(HALVES):
                hA, hB = p, p + 2
                for t in range(8):
                    emit_scores(i, hA, t)
                    emit_scores(i, hB, t)
                    if prev is not None:
                        pi, pp = prev
                        if t == 0:
                            ycur = yps.tile([33, S], F32, name="y", tag="y")
                            ysave = (
                                rcp.tile([32, S], BF16, name="ya", bufs=1),
                                rcp.tile([32, S], BF16, name="yb", bufs=1),
                            )
                            cur_rc = next_rc()
                            y_quarter(pi, pp, ycur, 0)
                        elif t == 1:
                            y_quarter(pi, pp, ycur, 1)
                            y_save(ycur, ysave[0], cur_rc, 0)
                        elif t == 2:
                            y_quarter(pi, pp, ycur, 2)
                        elif t == 3:
                            y_quarter(pi, pp, ycur, 3)
                            y_save(ycur, ysave[1], cur_rc, 32)
                        elif t == 4:
                            recip_pass(cur_rc)
                        elif t == 5:
                            mul_pass(pi, pp, ysave, cur_rc)
                    for th in fillers[j][t]:
                        th()
                prev = (i, p)

            # ---- tail --------------------------------------------------
            # last half's y/recip/mul -> conv1-br2 per m-pair with RS2
            # chunks pipelined -> h1 per k-chunk -> conv2 k-major with SBUF
            # accumulation (each RS chunk unlocks PE work immediately).
            rsl = [
                stgp.tile([128, 4, BR, W], BF16, name=f"rsl{i}", bufs=1)
                for i in range(2)
            ]
            racc = stgp.tile([128, 4, BR, W], BF16, name="racc", bufs=1)

            def preload_rs01():
                for i in range(2):
                    nc.sync.dma_start(
                        out=rsl[i],
                        in_=rsout[i][:].rearrange("(m p) r c -> p m r c", p=128),
                    )

            def add_rs01():
                nc.vector.tensor_add(racc, rsl[0], rsl[1])

            # NOTE gpsimd queue order: the RS enqueues must sit BEHIND the
            # broadcast/ship work they would otherwise head-of-line block.
            rc = next_rc()
            ylast = y_pass(2, 1, rc)
            recip_pass(rc)
            mul_pass(2, 1, ylast, rc)
            rs_branch(1)
            preload_rs01()
            # conv1-br2: m-tile order with ships ASAP; RS2 chunk a after its
            # two m-tiles shipped
            for m in range(4):
                conv1_block(2, m, 0)
                conv1_block(2, m, 1)
                if m == 1:
                    rs2_chunk(0)
            rs2_chunk(1)
            add_rs01()

            rsl2 = stgp.tile([128, 4, BR, W], BF16, name="rsl2", bufs=1)

            h1b = [stgp.tile([128, BR, HP], BF16, name=f"h1b{k}", bufs=1) for k in range(4)]
            for k in range(4):
                nc.vector.memset(h1b[k][:, :, 0:1], 0.0)
                nc.vector.memset(h1b[k][:, :, 33:34], 0.0)

            def h1_chunk(k):
                """h1b[k] = relu(bn1 * (racc[k] + rsl2[k]) + b) with edge
                halo masking."""
                nc.sync.dma_start(
                    out=rsl2[:, k],
                    in_=rsout2[k // 2][128 * (k % 2) : 128 * (k % 2) + 128],
                )
                nc.vector.tensor_add(rsl2[:, k], rsl2[:, k], racc[:, k])
                nc.scalar.activation(
                    out=h1b[k][:, :, 1:33],
                    in_=rsl2[:, k],
                    func=AF.Relu,
                    bias=avec[:, 4 + k : 5 + k],
                    scale=avec[:, k : k + 1],
                )
                # halo rows beyond the image edge must be exact zeros for
                # conv2's SAME padding (relu(bias) otherwise leaks in)
                nc.vector.tensor_scalar_mul(
                    h1b[k][:, 0:1, 1:33], h1b[k][:, 0:1, 1:33], hmask[:, 0:1]
                )
                nc.vector.tensor_scalar_mul(
                    h1b[k][:, 9:10, 1:33], h1b[k][:, 9:10, 1:33], hmask[:, 1:2]
                )

            # conv2 k-major: per (k, m) a 9-MM chain -> DVE-accumulate into
            # oacc; k01 runs while RS2 chunk 1 is in flight
            oacc = stgp.tile([128, 4, BW * W], F32, name="oacc", bufs=1)

            def conv2_k(k):
                h1_chunk(k)
                for m in range(4):
                    ps = smps.tile([128, 512], F32, name="cv2", tag="sm")
                    for dy in range(3):
                        for dx in range(3):
                            nc.tensor.matmul(
                                ps[:, 0 : BW * W],
                                lhsT=c2w_sb[k][m][:, dy * 3 + dx, :],
                                rhs=h1b[k][:, dy : dy + BW, dx : dx + 32],
                                start=(dy == 0 and dx == 0),
                                stop=(dy == 2 and dx == 2),
                            )
                    if k == 0:
                        nc.vector.tensor_copy(oacc[:, m], ps[:, 0 : BW * W])
                    else:
                        nc.vector.tensor_add(
                            oacc[:, m], oacc[:, m], ps[:, 0 : BW * W]
                        )

            for k in range(4):
                conv2_k(k)
            for m in range(4):
                nc.scalar.activation(
                    out=oacc[:, m],
                    in_=oacc[:, m],
                    func=AF.Relu,
                    bias=avec[:, 12 + m : 13 + m],
                    scale=avec[:, 8 + m : 9 + m],
                )
                nc.sync.dma_start(out=out_d[:, m], in_=oacc[:, m])

    nc.finalize()
    return nc


def _f(x):
    return np.ascontiguousarray(x, dtype=np.float32)


def _bf(x):
    return np.ascontiguousarray(np.asarray(x, dtype=np.float32).astype(ml_dtypes.bfloat16))


def prepare_core_inputs(inp):
    """Build the 8 per-core input dicts from the full-problem inputs."""
    inp = {k: np.asarray(v, dtype=np.float64) for k, v in inp.items()}
    x = inp["x"].reshape(B, C, S)
    xp = inp["x_prev"].reshape(B, C, S)
    xn = inp["x_next"].reshape(B, C, S)

    bn1s_full = inp["bn1g"] / np.sqrt(inp["bn1v"] + EPS)
    bn1b_full = inp["bn1b"] - inp["bn1m"] * bn1s_full
    bn2s_full = inp["bn2g"] / np.sqrt(inp["bn2v"] + EPS)
    bn2b_full = inp["bn2b"] - inp["bn2m"] * bn2s_full

    # conv2 weights: full, same for every rank: c2wT[k][m] = [128 in-part, 9, 128 out]
    c2wT = np.stack(
        [
            np.stack(
                [
                    inp["c2w"][128 * m : 128 * (m + 1), 128 * k : 128 * (k + 1)]
                    .transpose(1, 2, 3, 0)
                    .reshape(128, 9, 128)
                    for m in range(4)
                ]
            )
            for k in range(4)
        ]
    )
    avec = np.concatenate(
        [
            bn1s_full.reshape(4, 128).T,
            bn1b_full.reshape(4, 128).T,
            bn2s_full.reshape(4, 128).T,
            bn2b_full.reshape(4, 128).T,
        ],
        axis=1,
    )  # (128, 16)

    per_g = []
    for g in range(4):
        sl = slice(128 * g, 128 * (g + 1))
        wqT = np.stack(
            [
                np.stack([inp["Wq"][i][sl, 128 * k : 128 * (k + 1)].T for k in range(4)])
                for i in range(3)
            ]
        )
        wkT = np.stack(
            [
                np.stack([inp["Wk"][i][sl, 128 * k : 128 * (k + 1)].T for k in range(4)])
                for i in range(3)
            ]
        )
        bqv = np.stack([inp["bq"][i][sl] for i in range(3)], axis=1)
        bkv = np.stack([inp["bk"][i][sl] for i in range(3)], axis=1)

        att_s = np.stack(
            [inp["bng"][i][sl] / np.sqrt(inp["bnv"][i][sl] + EPS) for i in range(3)]
        )  # (3,128)
        xtb = np.stack(
            [
                inp["bnb"][i][sl] + (inp["bo"][i][sl] - inp["bnm"][i][sl]) * att_s[i]
                for i in range(3)
            ]
        )  # (3,128)

        wvo_rows = []
        wobv_row = []
        for i in range(3):
            for hl in range(4):
                hg = 4 * g + hl
                wv_h = inp["Wv"][i][32 * hg : 32 * (hg + 1), :]  # (32, 512)
                bv_h = inp["bv"][i][32 * hg : 32 * (hg + 1)]
                wo_h = inp["Wo"][i, hg]  # (32, 32)
                sc = att_s[i][32 * hl : 32 * (hl + 1)]  # (32,)
                wvo_rows.append(sc[:, None] * (wo_h @ wv_h))
                wobv_row.append(sc * (wo_h @ bv_h) + xtb[i][32 * hl : 32 * (hl + 1)])
        wvo_all = np.concatenate(wvo_rows, axis=0)  # (384, 512)
        wobv = np.concatenate(wobv_row)[None, :]  # (1, 384)
        wvoT = np.stack([wvo_all[:, 128 * k : 128 * (k + 1)].T for k in range(4)])

        c1wT = np.stack(
            [
                np.stack(
                    [
                        inp["c1w"][
                            128 * m : 128 * (m + 1),
                            512 * i + 128 * g : 512 * i + 128 * (g + 1),
                        ]
                        .transpose(1, 2, 3, 0)
                        .reshape(128, 9, 128)
                        for m in range(4)
                    ]
                )
                for i in range(3)
            ]
        )

        per_g.append(
            dict(
                wqT=_bf(wqT), wkT=_bf(wkT), wvoT=_bf(wvoT),
                wobv=_f(wobv), c1wT=_bf(c1wT), c2wT=_bf(c2wT),
                dvec=_f(np.concatenate([bqv, bkv], axis=1)),
                avec=_f(avec),
            )
        )

    in_maps = []
    for c in range(NCORES):
        b, g = c // 4, c % 4
        d = dict(per_g[g])
        d["x4"] = _bf(x[b].reshape(4, 128, S))
        d["oth"] = _bf(np.stack([xn[b].reshape(4, 128, S), xp[b].reshape(4, 128, S)]))
        hm = np.ones((128, 2), dtype=np.float32)
        if g == 0:
            hm[:, 0] = 0.0
        if g == 3:
            hm[:, 1] = 0.0
        d["hmask"] = hm
        in_maps.append(d)
    return in_maps


_NC_CACHE = {}


def get_nc():
    if "nc" not in _NC_CACHE:
        _NC_CACHE["nc"] = build_nc()
    return _NC_CACHE["nc"]


def assemble(results):
    out = np.zeros((B, C, H, W), dtype=np.float32)
    for c in range(NCORES):
        b, g = c // 4, c % 4
        o = results[c]["out"].reshape(128, 4, BW, W)
        for m in range(4):
            out[b, 128 * m : 128 * (m + 1), BW * g : BW * (g + 1), :] = o[:, m]
    return out


def kernel(**inputs):
    nc = get_nc()
    in_maps = prepare_core_inputs(inputs)
    res = run_bass_kernel_spmd(nc, in_maps, list(range(NCORES)))
    return assemble(res.results)


# revision 44
# speedup vs baseline: 1.2073x; 1.0135x over previous
"""CSAEncoder Trainium2 kernel v3: ACT-saturated attention + band-ReduceScatter convs.

Sharding (8 cores): 2 batch groups x 4 tensor ranks.  Core c: batch b = c // 4,
rank g = c % 4.

Attention (per core): heads [4g, 4g+4) of all 3 branches for batch b.
Processed as 6 "halves" (3 branches x head-pairs {0,2} / {1,3}):
  - exps stream t-major on ACT (the binding engine: 96 x [128,1024] exps
    ~ 110us).  2 rotating score psum tiles keep ACT fed.
  - per-half y-pass at the half boundary: 2-way column-tile-packed matmuls
    (M=33 incl. the ones-row denominator) at PE col-groups 0 and 64, emitted
    with a 2-t-slot lag into the next half so ACT never stalls.
  - reciprocals: DVE copies the two denominator rows to a compact [2,1024]
    tile; ACT does ln + exp(-x) (natural_log_exp_and_others table set);
    gpsimd broadcasts; DVE multiplies into the padded xt slab.

Convs: conv1 partials (contract own 128 ch/branch, full space) are staged
into 4 overlapping row-bands of 10 rows (8-row band + 1-row halo, zero rows
at the image edges) and ReduceScattered per branch: the RS chunk routing
delivers each rank ITS band fully reduced, halo included -- rank-dependent
band selection without breaking SPMD.  conv2 then runs fully local on the
band (all 512 out channels, N=256 matmuls), output is banded: core owns
out[b, :, 8g:8g+8, :].
"""

import os
import sys

import ml_dtypes
import numpy as np

for _p in ("/opt/trn_rl_repo",):
    if _p not in sys.path and os.path.isdir(_p):
        sys.path.insert(0, _p)

import concourse.bass as bass
import concourse.mybir as mybir
import concourse.tile as tile
from concourse import bacc
from concourse.bass_utils import run_bass_kernel_spmd

F32 = mybir.dt.float32
BF16 = mybir.dt.bfloat16
F8 = mybir.dt.float8e4
AF = mybir.ActivationFunctionType

# Restrict ACT table matching to the one set containing exp, ln, relu, copy
# so exactly one table load is emitted (see baseline kernel notes).
_ACT_KEEP_SET = "natural_log_exp_and_others"
_orig_get_act_tables = bacc.get_activation_tables


def _patched_get_act_tables(arch):
    tabs = _orig_get_act_tables(arch)
    return {n: (fns if n == _ACT_KEEP_SET else set()) for n, fns in tabs.items()}


bacc.get_activation_tables = _patched_get_act_tables

B, C, H, W, HEADS = 2, 512, 32, 32, 16
D = C // HEADS            # 32
S = H * W                 # 1024
EPS = 1e-5
ISQD = 1.0 / np.sqrt(D)
NCORES = 8
GROUPS = [[0, 1, 2, 3], [4, 5, 6, 7]]
HP = W + 2                # padded row stride for xt slabs (34)
BR = 10                   # band rows incl 1-row halo each side
BW = 8                    # band rows owned


def build_nc():
    nc = bacc.Bacc(None, target_bir_lowering=False)

    # ---- per-core external inputs -------------------------------------
    x4_d = nc.declare_dram_parameter("x4", [4, 128, S], BF16, isOutput=False)
    oth_d = nc.declare_dram_parameter("oth", [2, 4, 128, S], BF16, isOutput=False)
    wqT_d = nc.declare_dram_parameter("wqT", [3, 4, 128, 128], BF16, isOutput=False)
    wkT_d = nc.declare_dram_parameter("wkT", [3, 4, 128, 128], BF16, isOutput=False)
    wvoT_d = nc.declare_dram_parameter("wvoT", [4, 128, 384], BF16, isOutput=False)
    dvec_d = nc.declare_dram_parameter("dvec", [128, 6], F32, isOutput=False)
    wobv_d = nc.declare_dram_parameter("wobv", [1, 384], F32, isOutput=False)
    c1wT_d = nc.declare_dram_parameter("c1wT", [3, 4, 128, 9, 128], BF16, isOutput=False)
    # full conv2 weights: [k-chunk, m-chunk, 128 in-part, 9 taps, 128 out]
    c2wT_d = nc.declare_dram_parameter("c2wT", [4, 4, 128, 9, 128], BF16, isOutput=False)
    # avec: [bn1s(4) bn1b(4) bn2s(4) bn2b(4)] = [128, 16]
    avec_d = nc.declare_dram_parameter("avec", [128, 16], F32, isOutput=False)
    # hmask: [top-halo-valid, bottom-halo-valid] per rank (0.0 at image edge)
    hmask_d = nc.declare_dram_parameter("hmask", [128, 2], F32, isOutput=False)
    out_d = nc.declare_dram_parameter("out", [128, 4, BW * W], F32, isOutput=True)

    with tile.TileContext(nc) as tc:
        import contextlib

        ctx = contextlib.ExitStack()
        with ctx:
            const = ctx.enter_context(tc.tile_pool(name="const", bufs=1))
            kq = ctx.enter_context(tc.tile_pool(name="kq", bufs=1))
            xtp = ctx.enter_context(tc.tile_pool(name="xtp", bufs=1))
            rcp = ctx.enter_context(tc.tile_pool(name="rcp", bufs=2))
            stgp = ctx.enter_context(tc.tile_pool(name="stgp", bufs=1))
            scps = ctx.enter_context(tc.tile_pool(name="scps", bufs=2, space="PSUM"))
            yps = ctx.enter_context(tc.tile_pool(name="yps", bufs=1, space="PSUM"))
            smps = ctx.enter_context(tc.tile_pool(name="smps", bufs=2, space="PSUM"))
            dram = ctx.enter_context(tc.tile_pool(name="dram", bufs=1, space="DRAM"))

            # ---- activations + weights (DMA priority order) --------------
            x_sb = const.tile([128, 4, S], BF16)
            nc.sync.dma_start(out=x_sb, in_=x4_d[:].rearrange("k p s -> p k s"))
            wq_sb = const.tile([128, 3, 4, 128], BF16)
            wk_sb = const.tile([128, 3, 4, 128], BF16)
            nc.sync.dma_start(out=wk_sb, in_=wkT_d[:].rearrange("i k p f -> p i k f"))
            nc.sync.dma_start(out=wq_sb, in_=wqT_d[:].rearrange("i k p f -> p i k f"))
            othp = ctx.enter_context(tc.tile_pool(name="othp", bufs=1))
            oth1 = othp.tile([128, 4, S], BF16, name="oth")
            nc.sync.dma_start(out=oth1, in_=oth_d[0].rearrange("k p s -> p k s"))
            wvo_sb = const.tile([128, 4, 384], BF16)
            nc.sync.dma_start(out=wvo_sb, in_=wvoT_d[:].rearrange("k p f -> p k f"))

            # Small consts: DMA to staging, then re-own on the consuming
            # engine so consumers need no cross-engine const wait.
            dvec_st = const.tile([128, 6], F32)
            nc.gpsimd.dma_start(out=dvec_st, in_=dvec_d[:])
            wobv_st = const.tile([128, 384], F32)
            nc.gpsimd.dma_start(out=wobv_st, in_=wobv_d[:].partition_broadcast(128))
            avec_st = const.tile([128, 16], F32)
            nc.gpsimd.dma_start(out=avec_st, in_=avec_d[:])
            hmask_st = const.tile([128, 2], F32)
            nc.gpsimd.dma_start(out=hmask_st, in_=hmask_d[:])
            hmask = const.tile([128, 2], F32)
            nc.vector.tensor_copy(hmask, hmask_st)
            dvec = const.tile([128, 6], F32)
            nc.vector.tensor_copy(dvec, dvec_st)
            wobv_sb = const.tile([128, 384], F32)
            nc.vector.tensor_copy(wobv_sb, wobv_st)
            avec = const.tile([128, 16], F32)
            nc.scalar.activation(out=avec, in_=avec_st, func=AF.Copy)
            bqv_sb = dvec[:, 0:3]
            bkv_sb = dvec[:, 3:6]

            # xt (attention output) slabs, zero-padded 34x34
            xt_sl = []
            for i in range(3):
                t = xtp.tile([128, HP, HP], BF16, name=f"xt{i}")
                nc.vector.memset(t, 0.0)
                xt_sl.append(t)

            # Semaphore warmers
            warm = const.tile([128, 1], F32)
            nc.vector.tensor_copy(warm, dvec[:, 0:1])
            warm2 = const.tile([128, 1], F32)
            nc.scalar.activation(out=warm2, in_=warm, func=AF.Copy)

            # k/q per branch (with biases added); u in fp8 paired by t for
            # DoubleRow (layout [ki, ko(t-parity), branch, head, 36pad];
            # ones col at 32, last-dim padded to 36 so the ko byte-stride
            # (432) is 16-aligned)
            k_sb = kq.tile([128, 3, S], BF16)
            q_sb = kq.tile([128, 3, S], BF16)
            uT8 = [kq.tile([128, 2, 3, 4, 36], F8, name=f"uT8{tp}") for tp in range(4)]

            # exp outputs in fp8, t-paired [128, 2, S]: softmax probs + u in
            # fp8 self-normalize (denominator sums the same quantized p), so
            # accuracy cost is tiny while DoubleRow halves the y matmuls
            pt = ctx.enter_context(tc.tile_pool(name="pt", bufs=10))

            qsrc = [oth1, x_sb, None]

            def load_oth2():
                # same pool slot as oth1 (bufs=1): x_prev overwrites x_next
                # once q_proj(0) has consumed it
                oth2 = othp.tile([128, 4, S], BF16, name="oth")
                nc.sync.dma_start(out=oth2, in_=oth_d[1].rearrange("k p s -> p k s"))
                qsrc[2] = oth2

            def k_proj(i):
                for s in range(2):
                    ps = smps.tile([128, 512], F32, name="proj_ps", tag="sm")
                    for ks in range(4):
                        nc.tensor.matmul(
                            ps,
                            lhsT=wk_sb[:, i, ks, :],
                            rhs=x_sb[:, ks, 512 * s : 512 * (s + 1)],
                            start=(ks == 0),
                            stop=(ks == 3),
                        )
                    nc.vector.tensor_scalar_add(
                        k_sb[:, i, 512 * s : 512 * (s + 1)], ps, bkv_sb[:, i : i + 1]
                    )

            def q_proj(i):
                for s in range(2):
                    ps = smps.tile([128, 512], F32, name="proj_ps", tag="sm")
                    for ks in range(4):
                        nc.tensor.matmul(
                            ps,
                            lhsT=wq_sb[:, i, ks, :],
                            rhs=qsrc[i][:, ks, 512 * s : 512 * (s + 1)],
                            start=(ks == 0),
                            stop=(ks == 3),
                        )
                    nc.vector.tensor_scalar_add(
                        q_sb[:, i, 512 * s : 512 * (s + 1)], ps, bqv_sb[:, i : i + 1]
                    )

            def u_proj(t):
                u_ps = smps.tile([128, 512], F32, name="proj_ps", tag="sm")
                for ks in range(4):
                    nc.tensor.matmul(
                        u_ps[:, 0:384],
                        lhsT=x_sb[:, ks, 128 * t : 128 * (t + 1)],
                        rhs=wvo_sb[:, ks, :],
                        start=(ks == 0),
                        stop=(ks == 3),
                    )
                dst = uT8[t // 2][:, t % 2]
                nc.vector.memset(dst[:, :, :, 32:33], 1.0)
                nc.vector.tensor_add(
                    dst[:, :, :, 0:32],
                    u_ps[:, 0:384].rearrange("p (i h d) -> p i h d", i=3, h=4),
                    wobv_sb.rearrange("p (i h d) -> p i h d", i=3, h=4),
                )

            # ---- conv weights ------------------------------------------
            convw = ctx.enter_context(tc.tile_pool(name="convw", bufs=1))
            c1w_sb = [
                [convw.tile([128, 9, 128], BF16, name=f"c1w{i}_{m}") for m in range(4)]
                for i in range(3)
            ]
            c2w_sb = [
                [convw.tile([128, 9, 128], BF16, name=f"c2w{k}_{m}") for m in range(4)]
                for k in range(4)
            ]

            def load_conv1_w():
                for i in range(3):
                    for m in range(4):
                        nc.sync.dma_start(out=c1w_sb[i][m], in_=c1wT_d[i, m])

            def load_conv2_w():
                for k in range(4):
                    for m in range(4):
                        nc.sync.dma_start(out=c2w_sb[k][m], in_=c2wT_d[k, m])

            # ---- conv1 partial staging + band RS ------------------------
            # staged[m]: [128, 34, 32] bf16; row r+1 = image row r, rows 0/33
            # zero (SAME-pad at image top/bottom => also the RS halo pad).
            staged = [stgp.tile([128, HP, W], BF16, name=f"stg{m}") for m in range(4)]
            for m in range(4):
                nc.vector.memset(staged[m][:, 0:1, :], 0.0)
                nc.vector.memset(staged[m][:, 33:34, :], 0.0)

            rsin = [dram.tile([4, 512, BR, W], BF16, name=f"rsin{i}") for i in range(2)]
            rsout = [dram.tile([512, BR, W], BF16, name=f"rsout{i}") for i in range(2)]
            # branch 2: two half-channel RS chunks (pipelined tail)
            rsin2 = [dram.tile([4, 256, BR, W], BF16, name=f"rsin2_{a}") for a in range(2)]
            rsout2 = [dram.tile([256, BR, W], BF16, name=f"rsout2_{a}") for a in range(2)]

            def conv1_block(i, m, n):
                """Partial conv1 for branch i, out m-tile, spatial half n ->
                staged[m] (bf16)."""
                ps = smps.tile([128, 512], F32, name="cv", tag="sm")
                for dy in range(3):
                    for dx in range(3):
                        nc.tensor.matmul(
                            ps,
                            lhsT=c1w_sb[i][m][:, dy * 3 + dx, :],
                            rhs=xt_sl[i][:, 16 * n + dy : 16 * n + dy + 16, dx : dx + 32],
                            start=(dy == 0 and dx == 0),
                            stop=(dy == 2 and dx == 2),
                        )
                nc.vector.tensor_copy(
                    staged[m][:, 1 + 16 * n : 17 + 16 * n, :],
                    ps.rearrange("p (a b) -> p a b", b=32),
                )
                if n == 1:
                    # ship m-tile into the 4 overlapping band-chunks
                    for jj in range(4):
                        if i < 2:
                            dst = rsin[i][jj, 128 * m : 128 * (m + 1), :, :]
                        else:
                            dst = rsin2[m // 2][jj, 128 * (m % 2) : 128 * (m % 2) + 128]
                        nc.gpsimd.dma_start(
                            out=dst, in_=staged[m][:, 8 * jj : 8 * jj + BR, :]
                        )

            def rs_branch(i):
                nc.gpsimd.collective_compute(
                    "ReduceScatter",
                    mybir.AluOpType.add,
                    replica_groups=GROUPS,
                    ins=[rsin[i][:]],
                    outs=[rsout[i][:]],
                )

            def rs2_chunk(a):
                nc.gpsimd.collective_compute(
                    "ReduceScatter",
                    mybir.AluOpType.add,
                    replica_groups=GROUPS,
                    ins=[rsin2[a][:]],
                    outs=[rsout2[a][:]],
                )

            # ---- attention halves ---------------------------------------
            # halves j = 0..5: branch i = j // 2, pair p = j % 2,
            # heads (p, p + 2); y col-groups at rows 0 and 64.
            ptt = {}

            def emit_scores(i, h, t):
                sc = scps.tile([128, S], F32, name="sc", tag="sc")
                p0 = 32 * h
                for s in range(2):
                    nc.tensor.matmul(
                        sc[:, 512 * s : 512 * (s + 1)],
                        lhsT=k_sb[p0 : p0 + 32, i, 128 * t : 128 * (t + 1)],
                        rhs=q_sb[p0 : p0 + 32, i, 512 * s : 512 * (s + 1)],
                        start=True,
                        stop=True,
                        tile_position=(p0, 0),
                    )
                if t % 2 == 0:
                    ptt[(i, h, t // 2)] = pt.tile([128, 2, S], F8, name="ptt")
                nc.scalar.activation(
                    out=ptt[(i, h, t // 2)][:, t % 2, :],
                    in_=sc,
                    func=AF.Exp,
                    scale=float(ISQD),
                )

            def y_quarter(i, p, y, q):
                """Quarter q (0..3) of the y chains for heads (p, p+2).
                Heads run SEQUENTIALLY into rows 0:33 of the same psum tile
                (on HW a matmul `start` zeroes the whole 2KB psum bank, so a
                second head may only start after the first head's rows are
                copied out -- see y_save).  fp8 DoubleRow contracts both
                t-chunks of a pair per matmul: q0/q1 = head p tp{0,1}/{2,3},
                q2/q3 = head p+2."""
                h = p if q < 2 else p + 2
                for tp in (2 * (q % 2), 2 * (q % 2) + 1):
                    for s in range(2):
                        nc.tensor.matmul(
                            y[0:33, 512 * s : 512 * (s + 1)],
                            lhsT=uT8[tp][:, :, i, h, 0:33],
                            rhs=ptt[(i, h, tp)][:, :, 512 * s : 512 * (s + 1)],
                            start=(tp == 0),
                            stop=(tp == 3),
                            perf_mode=mybir.MatmulPerfMode.DoubleRow,
                        )

            def y_save(y, ysb, rc, rr):
                # stage the finished head's y rows (bf16) + denominator row
                # (f32, for the ln) to SBUF before the next head's chain
                # re-starts (and bank-zeroes) the psum banks
                nc.vector.tensor_copy(ysb, y[0:32, :])
                nc.vector.tensor_copy(rc[rr : rr + 1, :], y[32:33, :])

            def y_pass(i, p, rc):
                y = yps.tile([33, S], F32, name="y", tag="y")
                ya = rcp.tile([32, S], BF16, name="ya", bufs=1)
                yb = rcp.tile([32, S], BF16, name="yb", bufs=1)
                y_quarter(i, p, y, 0)
                y_quarter(i, p, y, 1)
                y_save(y, ya, rc, 0)
                y_quarter(i, p, y, 2)
                y_quarter(i, p, y, 3)
                y_save(y, yb, rc, 32)
                return (ya, yb)

            # Two persistent rc buffers (alternating per half).  Rows 0/32
            # hold the two denominators; rows 1-31 are pre-set to 1.0 so the
            # batched [33, S] ln/exp reads only defined data (free-dim size
            # drives ACT cost, the extra partitions are free).
            rc_bufs = [rcp.tile([33, S], F32, name=f"rcb{a}", bufs=1) for a in range(2)]
            for a in range(2):
                nc.vector.memset(rc_bufs[a], 1.0)
            rc_idx = [0]

            def next_rc():
                rc = rc_bufs[rc_idx[0] % 2]
                rc_idx[0] += 1
                return rc

            def recip_pass(rc):
                nc.scalar.activation(out=rc, in_=rc, func=AF.Ln)
                nc.scalar.activation(out=rc, in_=rc, func=AF.Exp, scale=-1.0)
                return rc

            def mul_pass(i, p, yt, rc):
                ya, yb = yt
                hA, hB = p, p + 2
                for (h, ysb, rr) in ((hA, ya, 0), (hB, yb, 32)):
                    src = rc[rr : rr + 1, :]
                    if rr != 0:
                        # partition_broadcast reads garbage from non-zero
                        # base partitions on HW: stage through a base-0 tile
                        rc2 = rcp.tile([1, S], F32, name="rc2")
                        nc.vector.tensor_copy(rc2, src)
                        src = rc2[:]
                    rcb = rcp.tile([32, S], F32, name="rcbb")
                    nc.gpsimd.partition_broadcast(rcb, src)
                    nc.vector.tensor_mul(
                        xt_sl[i][32 * h : 32 * h + 32, 1:33, 1:33],
                        ysb[0:32, :].rearrange("p (a b) -> p a b", b=32),
                        rcb.rearrange("p (a b) -> p a b", b=32),
                    )

            # ---- the pipelined emission ---------------------------------
            # Halves j = 0..5 = (branch j//2, pair j%2).  Slot structure of
            # half j (8 t-slots): each slot emits the 2 heads' score MMs and
            # their exps; the PREVIOUS half's trailing work is injected in
            # quarter-granular pieces so the PE FIFO never blocks scores:
            #   slots 0-3: one y-quarter each; slot 4: recip; slot 5: muls;
            #   slots 2-7: filler thunks (projections / conv1 / RS issues);
            # fillers must be data-ready at their FIFO position (conv1 of
            # branch b only after branch b's muls are emitted).
            HALVES = [(j // 2, j % 2) for j in range(6)]

            c1q = {
                i: [(lambda i=i, m=m, n=n: conv1_block(i, m, n))
                    for m in range(4) for n in range(2)]
                for i in range(3)
            }
            # fillers[j][t] = list of thunks for half j, slot t
            fillers = {j: {t: [] for t in range(8)} for j in range(6)}
            for t in range(4):
                fillers[0][2 + t] = [lambda t=t: u_proj(t)]
            fillers[0][6] = [lambda: u_proj(4), lambda: u_proj(5)]
            fillers[0][7] = [lambda: u_proj(6), lambda: u_proj(7)]
            fillers[1][3] = [load_conv1_w]
            fillers[1][6] = [lambda: k_proj(1)]
            fillers[1][7] = [lambda: q_proj(1)]
            fillers[2][2] = [lambda: k_proj(2)]
            fillers[2][3] = [lambda: q_proj(2)]
            # xt0 complete after half-2 slot-5 muls -> conv1 br0 from slot 6
            fillers[2][6] = c1q[0][0:1]
            fillers[2][7] = c1q[0][1:2]
            for t in range(6):
                fillers[3][2 + t] = c1q[0][2 + t : 3 + t]
            fillers[4][2] = [lambda: rs_branch(0), load_conv2_w]
            fillers[4][6] = c1q[1][0:1]
            fillers[4][7] = c1q[1][1:2]
            for t in range(6):
                fillers[5][2 + t] = c1q[1][2 + t : 3 + t]

            k_proj(0)
            q_proj(0)
            load_oth2()

            prev = None  # (i, p, ytile) trailing from previous half

            for j, (i, p) in enumerate(HALVES):
                hA, hB = p, p + 2
                for t in range(8):
                    emit_scores(i, hA, t)
                    emit_scores(i, hB, t)
                    if prev is not None:
                        pi, pp = prev
                        if t == 0:
                            ycur = yps.tile([33, S], F32, name="y", tag="y")
                            ysave = (
                                rcp.tile([32, S], BF16, name="ya", bufs=1),
                                rcp.tile([32, S], BF16, name="yb", bufs=1),
                            )
                            cur_rc = next_rc()
                            y_quarter(pi, pp, ycur, 0)
                        elif t == 1:
                            y_quarter(pi, pp, ycur, 1)
                            y_save(ycur, ysave[0], cur_rc, 0)
                        elif t == 2:
                            y_quarter(pi, pp, ycur, 2)
                        elif t == 3:
                            y_quarter(pi, pp, ycur, 3)
                            y_save(ycur, ysave[1], cur_rc, 32)
                        elif t == 4:
                            recip_pass(cur_rc)
                        elif t == 5:
                            mul_pass(pi, pp, ysave, cur_rc)
                    for th in fillers[j][t]:
                        th()
                prev = (i, p)

            # ---- tail --------------------------------------------------
            # last half's y/recip/mul -> conv1-br2 per m-pair with RS2
            # chunks pipelined -> h1 per k-chunk -> conv2 k-major with SBUF
            # accumulation (each RS chunk unlocks PE work immediately).
            rsl = [
                stgp.tile([128, 4, BR, W], BF16, name=f"rsl{i}", bufs=1)
                for i in range(2)
            ]
            racc = stgp.tile([128, 4, BR, W], BF16, name="racc", bufs=1)

            def preload_rs01():
                for i in range(2):
                    nc.sync.dma_start(
                        out=rsl[i],
                        in_=rsout[i][:].rearrange("(m p) r c -> p m r c", p=128),
                    )

            def add_rs01():
                nc.vector.tensor_add(racc, rsl[0], rsl[1])

            # NOTE gpsimd queue order: the RS enqueues must sit BEHIND the
            # broadcast/ship work they would otherwise head-of-line block.
            rc = next_rc()
            ylast = y_pass(2, 1, rc)
            recip_pass(rc)
            mul_pass(2, 1, ylast, rc)
            rs_branch(1)
            preload_rs01()
            # conv1-br2: m-tile order with ships ASAP; RS2 chunk a after its
            # two m-tiles shipped
            for m in range(4):
                conv1_block(2, m, 0)
                conv1_block(2, m, 1)
                if m == 1:
                    rs2_chunk(0)
            rs2_chunk(1)
            add_rs01()

            rsl2 = stgp.tile([128, 4, BR, W], BF16, name="rsl2", bufs=1)

            h1b = [stgp.tile([128, BR, HP], BF16, name=f"h1b{k}", bufs=1) for k in range(4)]
            for k in range(4):
                nc.vector.memset(h1b[k][:, :, 0:1], 0.0)
                nc.vector.memset(h1b[k][:, :, 33:34], 0.0)

            def h1_chunk(k):
                """h1b[k] = relu(bn1 * (racc[k] + rsl2[k]) + b) with edge
                halo masking."""
                nc.sync.dma_start(
                    out=rsl2[:, k],
                    in_=rsout2[k // 2][128 * (k % 2) : 128 * (k % 2) + 128],
                )
                nc.vector.tensor_add(rsl2[:, k], rsl2[:, k], racc[:, k])
                nc.scalar.activation(
                    out=h1b[k][:, :, 1:33],
                    in_=rsl2[:, k],
                    func=AF.Relu,
                    bias=avec[:, 4 + k : 5 + k],
                    scale=avec[:, k : k + 1],
                )
                # halo rows beyond the image edge must be exact zeros for
                # conv2's SAME padding (relu(bias) otherwise leaks in)
                nc.vector.tensor_scalar_mul(
                    h1b[k][:, 0:1, 1:33], h1b[k][:, 0:1, 1:33], hmask[:, 0:1]
                )
                nc.vector.tensor_scalar_mul(
                    h1b[k][:, 9:10, 1:33], h1b[k][:, 9:10, 1:33], hmask[:, 1:2]
                )

            # conv2 k-major: per (k, m) a 9-MM chain -> DVE-accumulate into
            # oacc; k01 runs while RS2 chunk 1 is in flight
            oacc = stgp.tile([128, 4, BW * W], F32, name="oacc", bufs=1)

            def conv2_k(k):
                h1_chunk(k)
                for m in range(4):
                    ps = smps.tile([128, 512], F32, name="cv2", tag="sm")
                    for dy in range(3):
                        for dx in range(3):
                            nc.tensor.matmul(
                                ps[:, 0 : BW * W],
                                lhsT=c2w_sb[k][m][:, dy * 3 + dx, :],
                                rhs=h1b[k][:, dy : dy + BW, dx : dx + 32],
                                start=(dy == 0 and dx == 0),
                                stop=(dy == 2 and dx == 2),
                            )
                    if k == 0:
                        nc.vector.tensor_copy(oacc[:, m], ps[:, 0 : BW * W])
                    else:
                        nc.vector.tensor_add(
                            oacc[:, m], oacc[:, m], ps[:, 0 : BW * W]
                        )

            for k in range(4):
                conv2_k(k)
            for m in range(4):
                nc.scalar.activation(
                    out=oacc[:, m],
                    in_=oacc[:, m],
                    func=AF.Relu,
                    bias=avec[:, 12 + m : 13 + m],
                    scale=avec[:, 8 + m : 9 + m],
                )
                nc.sync.dma_start(out=out_d[:, m], in_=oacc[:, m])

    nc.finalize()
    return nc


def _f(x):
    return np.ascontiguousarray(x, dtype=np.float32)


def _bf(x):
    return np.ascontiguousarray(np.asarray(x, dtype=np.float32).astype(ml_dtypes.bfloat16))


def prepare_core_inputs(inp):
    """Build the 8 per-core input dicts from the full-problem inputs."""
    inp = {k: np.asarray(v, dtype=np.float64) for k, v in inp.items()}
    x = inp["x"].reshape(B, C, S)
    xp = inp["x_prev"].reshape(B, C, S)
    xn = inp["x_next"].reshape(B, C, S)

    bn1s_full = inp["bn1g"] / np.sqrt(inp["bn1v"] + EPS)
    bn1b_full = inp["bn1b"] - inp["bn1m"] * bn1s_full
    bn2s_full = inp["bn2g"] / np.sqrt(inp["bn2v"] + EPS)
    bn2b_full = inp["bn2b"] - inp["bn2m"] * bn2s_full

    # conv2 weights: full, same for every rank: c2wT[k][m] = [128 in-part, 9, 128 out]
    c2wT = np.stack(
        [
            np.stack(
                [
                    inp["c2w"][128 * m : 128 * (m + 1), 128 * k : 128 * (k + 1)]
                    .transpose(1, 2, 3, 0)
                    .reshape(128, 9, 128)
                    for m in range(4)
                ]
            )
            for k in range(4)
        ]
    )
    avec = np.concatenate(
        [
            bn1s_full.reshape(4, 128).T,
            bn1b_full.reshape(4, 128).T,
            bn2s_full.reshape(4, 128).T,
            bn2b_full.reshape(4, 128).T,
        ],
        axis=1,
    )  # (128, 16)

    per_g = []
    for g in range(4):
        sl = slice(128 * g, 128 * (g + 1))
        wqT = np.stack(
            [
                np.stack([inp["Wq"][i][sl, 128 * k : 128 * (k + 1)].T for k in range(4)])
                for i in range(3)
            ]
        )
        wkT = np.stack(
            [
                np.stack([inp["Wk"][i][sl, 128 * k : 128 * (k + 1)].T for k in range(4)])
                for i in range(3)
            ]
        )
        bqv = np.stack([inp["bq"][i][sl] for i in range(3)], axis=1)
        bkv = np.stack([inp["bk"][i][sl] for i in range(3)], axis=1)

        att_s = np.stack(
            [inp["bng"][i][sl] / np.sqrt(inp["bnv"][i][sl] + EPS) for i in range(3)]
        )  # (3,128)
        xtb = np.stack(
            [
                inp["bnb"][i][sl] + (inp["bo"][i][sl] - inp["bnm"][i][sl]) * att_s[i]
                for i in range(3)
            ]
        )  # (3,128)

        wvo_rows = []
        wobv_row = []
        for i in range(3):
            for hl in range(4):
                hg = 4 * g + hl
                wv_h = inp["Wv"][i][32 * hg : 32 * (hg + 1), :]  # (32, 512)
                bv_h = inp["bv"][i][32 * hg : 32 * (hg + 1)]
                wo_h = inp["Wo"][i, hg]  # (32, 32)
                sc = att_s[i][32 * hl : 32 * (hl + 1)]  # (32,)
                wvo_rows.append(sc[:, None] * (wo_h @ wv_h))
                wobv_row.append(sc * (wo_h @ bv_h) + xtb[i][32 * hl : 32 * (hl + 1)])
        wvo_all = np.concatenate(wvo_rows, axis=0)  # (384, 512)
        wobv = np.concatenate(wobv_row)[None, :]  # (1, 384)
        wvoT = np.stack([wvo_all[:, 128 * k : 128 * (k + 1)].T for k in range(4)])

        c1wT = np.stack(
            [
                np.stack(
                    [
                        inp["c1w"][
                            128 * m : 128 * (m + 1),
                            512 * i + 128 * g : 512 * i + 128 * (g + 1),
                        ]
                        .transpose(1, 2, 3, 0)
                        .reshape(128, 9, 128)
                        for m in range(4)
                    ]
                )
                for i in range(3)
            ]
        )

        per_g.append(
            dict(
                wqT=_bf(wqT), wkT=_bf(wkT), wvoT=_bf(wvoT),
                wobv=_f(wobv), c1wT=_bf(c1wT), c2wT=_bf(c2wT),
                dvec=_f(np.concatenate([bqv, bkv], axis=1)),
                avec=_f(avec),
            )
        )

    in_maps = []
    for c in range(NCORES):
        b, g = c // 4, c % 4
        d = dict(per_g[g])
        d["x4"] = _bf(x[b].reshape(4, 128, S))
        d["oth"] = _bf(np.stack([xn[b].reshape(4, 128, S), xp[b].reshape(4, 128, S)]))
        hm = np.ones((128, 2), dtype=np.float32)
        if g == 0:
            hm[:, 0] = 0.0
        if g == 3:
            hm[:, 1] = 0.0
        d["hmask"] = hm
        in_maps.append(d)
    return in_maps


_NC_CACHE = {}


def get_nc():
    if "nc" not in _NC_CACHE:
        _NC_CACHE["nc"] = build_nc()
    return _NC_CACHE["nc"]


def assemble(results):
    out = np.zeros((B, C, H, W), dtype=np.float32)
    for c in range(NCORES):
        b, g = c // 4, c % 4
        o = results[c]["out"].reshape(128, 4, BW, W)
        for m in range(4):
            out[b, 128 * m : 128 * (m + 1), BW * g : BW * (g + 1), :] = o[:, m]
    return out


def kernel(**inputs):
    nc = get_nc()
    in_maps = prepare_core_inputs(inputs)
    res = run_bass_kernel_spmd(nc, in_maps, list(range(NCORES)))
    return assemble(res.results)
